# revision 2
# baseline (speedup 1.0000x reference)
"""GQA sparse-attention kernel for 8 Trainium2 NeuronCores — v2.

Sharding: data-parallel over batch (2) x sequence-parallel over query rows
(rows j::4 interleaved). No collectives.

v2 changes vs baseline:
  - exact causal trimming of sim/PV/exp/mask work: per s-block k (128 wide),
    only query columns n >= 32k are computed (local row n <-> global 4n+j).
    Blocks are packed into PSUM tiles at bank-aligned offsets so exp/mask run
    on a few contiguous spans. sim+PV columns drop from 6144 to 4352 per head.
  - all-masked-row fallback moved to host (tiny numpy fixup on exact rows);
    removes the em_* device pipeline.
  - all input DMAs issued upfront (SP queue for the p1/p2 critical path, ACT
    queue for masks/p4 weights); adj arrives as prebuilt f16 mask tiles.
  - per-head tail: PE broadcast of denominator + DVE reciprocal + multiply.
  - software-pipelined per-head issue order (sim runs 2-3 tiles ahead of PV).
"""

import os
import sys

import numpy as np

for _p in ("/opt/trn_rl_repo", "/root/.axon_site/_ro/trn_rl_repo"):
    if os.path.isdir(_p) and _p not in sys.path:
        sys.path.insert(0, _p)

B, N, E = 2, 2048, 1024
HQ, HK, D = 16, 4, 64
G = HQ // HK          # 4 query heads per kv head
KVE = HK * D          # 256
NL = N // 4           # 512 local query rows per core
SB = N // 128         # 16 s-blocks
EC = E // 128         # 8 embedding chunks
LN_EPS = 1e-5

# s-block layout: block k -> (tile index, col offset, n0, width)
# tile A..E are [128,1024] (2 PSUM banks), F is [128,512] (1 bank)
BLK = [
    (0, 0, 0, 512), (0, 512, 32, 480),
    (1, 0, 64, 448), (1, 512, 96, 416),
    (2, 0, 128, 384), (2, 512, 160, 352),
    (3, 0, 192, 320), (3, 512, 224, 288),
    (4, 0, 256, 256), (4, 256, 288, 224), (4, 512, 320, 192), (4, 704, 352, 160),
    (5, 0, 384, 128), (5, 128, 416, 96), (5, 224, 448, 64), (5, 288, 480, 32),
]
# contiguous exp/mask spans per tile: (col_lo, col_hi)
SPANS = [
    [(0, 992)],
    [(0, 448), (512, 928)],
    [(0, 384), (512, 864)],
    [(0, 320), (512, 800)],
    [(0, 480), (512, 864)],
    [(0, 320)],
]
TILE_W = [1024, 1024, 1024, 1024, 1024, 512]
TILE_BLOCKS = [[0, 1], [2, 3], [4, 5], [6, 7], [8, 9, 10, 11], [12, 13, 14, 15]]

_PROG_CACHE = {}


def build_program():
    import concourse.mybir as mybir
    import concourse.tile as tile
    from concourse import bacc

    dt = mybir.dt
    f32, f32r, f16, i32 = dt.float32, dt.float32r, dt.float16, dt.int32
    AF = mybir.ActivationFunctionType
    OP = mybir.AluOpType
    AX = mybir.AxisListType

    nc = bacc.Bacc("TRN2", target_bir_lowering=False, debug=False)

    def din(name, shape, dtp=f32):
        return nc.dram_tensor(name, shape, dtp, kind="ExternalInput").ap()

    xqT = din("xqT", [E, NL], f16)
    xkT = din("xkT", [E, N], f16)
    xvT = din("xvT", [E, N], f16)
    WqT = din("WqT", [E, E], f16)          # pre-scaled by 1/8 on host
    WkT = din("WkT", [E, KVE], f16)
    WvT = din("WvT", [E, KVE], f16)
    WoT = din("WoT", [E, E], f16)
    bq2d = din("bq2d", [EC, 128])          # bq/8
    bk2d = din("bk2d", [2, 128])
    bv2d = din("bv2d", [2, 128])
    bo1 = din("bo1", [1, E], f16)
    lng = din("lng", [EC, 128])
    lnb = din("lnb", [EC, 128])
    ones1 = din("ones1", [1, 128], f32r)
    ones1h = din("ones1h", [1, 128], f16)
    masks_in = [din(f"mask{t}", [128, TILE_W[t]], f16) for t in range(6)]
    y = nc.dram_tensor("y", [NL, E], f32, kind="ExternalOutput").ap()

    with tile.TileContext(nc) as tc, nc.allow_low_precision(
            "f16/f32r operands for PE fast-path matmuls are intentional"):
        with (
            tc.tile_pool(name="const", bufs=1) as pc,
            tc.tile_pool(name="persist", bufs=1) as pp,
            tc.tile_pool(name="bigx", bufs=1) as pbx,
        ):
            # ---- upfront DMA prefetch ----
            # SP queue: p1 then p2 critical path (few big DMAs)
            wq_all = pc.tile([128, EC * E], f16, tag="wq_all")
            wq_sb = [wq_all[:, e * E:(e + 1) * E] for e in range(EC)]
            nc.sync.dma_start(
                wq_all[:].rearrange("p (e c) -> p e c", e=EC),
                WqT.rearrange("(e p) c -> p e c", e=EC))
            xq_all = pc.tile([128, EC * NL], f16, tag="xq_all")
            xq_sb = [xq_all[:, e * NL:(e + 1) * NL] for e in range(EC)]
            nc.sync.dma_start(
                xq_all[:].rearrange("p (e c) -> p e c", e=EC),
                xqT.rearrange("(e p) c -> p e c", e=EC))
            xk_all = pbx.tile([128, EC * N], f16, tag="xk_all")
            xv_all = pbx.tile([128, EC * N], f16, tag="xv_all")
            xk_sb = [xk_all[:, e * N:(e + 1) * N] for e in range(EC)]
            xv_sb = [xv_all[:, e * N:(e + 1) * N] for e in range(EC)]
            for st in range(4):
                ssl = slice(st * 512, (st + 1) * 512)
                nc.sync.dma_start(
                    xk_all[:].rearrange("p (e c) -> p e c", e=EC)[:, :, ssl],
                    xkT.rearrange("(e p) c -> p e c", e=EC)[:, :, ssl])
                nc.sync.dma_start(
                    xv_all[:].rearrange("p (e c) -> p e c", e=EC)[:, :, ssl],
                    xvT.rearrange("(e p) c -> p e c", e=EC)[:, :, ssl])
            # ACT queue: k/v weights, small consts, masks, p4 weights
            wk_all = pc.tile([128, EC * KVE], f16, tag="wk_all")
            wv_all = pc.tile([128, EC * KVE], f16, tag="wv_all")
            wk_sb = [wk_all[:, e * KVE:(e + 1) * KVE] for e in range(EC)]
            wv_sb = [wv_all[:, e * KVE:(e + 1) * KVE] for e in range(EC)]
            nc.scalar.dma_start(
                wk_all[:].rearrange("p (e c) -> p e c", e=EC),
                WkT.rearrange("(e p) c -> p e c", e=EC))
            nc.scalar.dma_start(
                wv_all[:].rearrange("p (e c) -> p e c", e=EC),
                WvT.rearrange("(e p) c -> p e c", e=EC))

            mask_sb = [pp.tile([128, TILE_W[t]], f16, tag=f"mk{t}", name=f"mk{t}")
                       for t in range(6)]
            for t in range(6):
                nc.scalar.dma_start(mask_sb[t][:], masks_in[t])
            bq_sb = [pc.tile([128, 1], f32, tag=f"bq{m}", name=f"bq{m}") for m in range(EC)]
            for e in range(EC):
                nc.scalar.dma_start(bq_sb[e][:], bq2d[e:e + 1, :])
            bk_sb = [pc.tile([128, 1], f32, tag=f"bk{m}", name=f"bk{m}") for m in range(2)]
            bv_sb = [pc.tile([128, 1], f32, tag=f"bv{m}", name=f"bv{m}") for m in range(2)]
            for m in range(2):
                nc.scalar.dma_start(bk_sb[m][:], bk2d[m:m + 1, :])
                nc.scalar.dma_start(bv_sb[m][:], bv2d[m:m + 1, :])
            ones_k1 = pc.tile([1, 128], f32r, tag="ones_k1")
            nc.scalar.dma_start(ones_k1[:], ones1)
            ones_m1 = pc.tile([128, 1], f16, tag="ones_m1")
            nc.scalar.dma_start(ones_m1[:], ones1h)
            ones_k1h = pc.tile([1, 128], f16, tag="ones_k1h")
            nc.scalar.dma_start(ones_k1h[:], ones1h)
            lng_sb = [pp.tile([128, 1], f32, tag=f"lng{e}", name=f"lng{e}") for e in range(EC)]
            lnb_sb = [pp.tile([128, 1], f32, tag=f"lnb{e}", name=f"lnb{e}") for e in range(EC)]
            for e in range(EC):
                nc.scalar.dma_start(lng_sb[e][:], lng[e:e + 1, :])
                nc.scalar.dma_start(lnb_sb[e][:], lnb[e:e + 1, :])
            bo_sb = pp.tile([1, E], f16, tag="bo", name="bo")
            nc.scalar.dma_start(bo_sb[:], bo1)
            wo_all = pp.tile([128, EC * E], f16, tag="wo_all", name="wo_all")
            wo_sb = [wo_all[:, e * E:(e + 1) * E] for e in range(EC)]
            nc.scalar.dma_start(
                wo_all[:].rearrange("p (e c) -> p e c", e=EC),
                WoT.rearrange("(e p) c -> p e c", e=EC))

            ident = pc.tile([128, 128], f16, tag="ident")
            from concourse.masks import make_identity
            make_identity(nc, ident[:])
            eps_c = pc.tile([1, 1], f32, tag="eps_c")
            nc.gpsimd.memset(eps_c[:], LN_EPS)
            ones64r = pc.tile([65, 64], f32r, tag="ones64r")
            nc.scalar.dma_start(ones64r[64:65, :], ones1[:, 0:64])

            # persistent activation tiles
            kT_sb = [pp.tile([128, N], f16, tag=f"kt{m}", name=f"kt{m}") for m in range(2)]
            v_ext = [pp.tile([128, 4 * 128], f16, tag=f"vx{k}", name=f"vx{k}") for k in range(SB)]
            qp_sb = [pp.tile([128, NL], f16, tag=f"qp{m}", name=f"qp{m}") for m in range(EC)]
            attT = [pp.tile([128, NL], f16, tag=f"at{e}", name=f"at{e}") for e in range(EC)]
            _EVEN = [0, 1, 2, 3, 8, 9, 10, 11]    # heads whose kv head is even
            _ODD = [4, 5, 6, 7, 12, 13, 14, 15]

            def _qslot(g):
                if (g // G) % 2 == 0:
                    return _EVEN.index(g), 0
                return _ODD.index(g), 1

            # ---------------- phase 1: q projection ----------------
            with tc.tile_pool(name="psq", bufs=2, space="PSUM") as psq:
                for mt2 in range(EC // 2):
                    psA = psq.tile([128, NL], f32, tag="psqA", name="psqA")
                    psB = psq.tile([128, NL], f32, tag="psqB", name="psqB")
                    for e in range(EC):
                        for mt, ps in ((2 * mt2, psA), (2 * mt2 + 1, psB)):
                            nc.tensor.matmul(
                                ps[:], wq_sb[e][:, mt * 128:(mt + 1) * 128],
                                xq_sb[e][:], start=(e == 0), stop=(e == EC - 1))
                    for mt, ps in ((2 * mt2, psA), (2 * mt2 + 1, psB)):
                        for t in range(2):
                            g = 2 * mt + t
                            ti, slot = _qslot(g)
                            nc.scalar.activation(
                                qp_sb[ti][slot * 64:(slot + 1) * 64, :],
                                ps[t * 64:(t + 1) * 64, :], AF.Identity,
                                bias=bq_sb[mt][t * 64:(t + 1) * 64, :], scale=1.0)

            # ---------------- phase 2: k/v projections ----------------
            with (
                tc.tile_pool(name="vt", bufs=2) as pvt,
                tc.tile_pool(name="pskv", bufs=2, space="PSUM") as pskv,
                tc.tile_pool(name="pst", bufs=2, space="PSUM") as pst,
            ):
                for st in range(4):  # s-tiles of 512
                    sl = slice(st * 512, (st + 1) * 512)
                    for mt in range(2):
                        psk = pskv.tile([128, 512], f32, tag="psk")
                        psv = pskv.tile([128, 512], f32, tag="psv")
                        for e in range(EC):
                            nc.tensor.matmul(
                                psk[:], wk_sb[e][:, mt * 128:(mt + 1) * 128],
                                xk_sb[e][:, sl], start=(e == 0), stop=(e == EC - 1))
                            nc.tensor.matmul(
                                psv[:], wv_sb[e][:, mt * 128:(mt + 1) * 128],
                                xv_sb[e][:, sl], start=(e == 0), stop=(e == EC - 1))
                        nc.scalar.activation(kT_sb[mt][:, sl], psk[:], AF.Identity,
                                             bias=bk_sb[mt][:], scale=1.0)
                        vt = pvt.tile([128, 512], f16, tag="vt")
                        nc.scalar.activation(vt[:], psv[:], AF.Identity,
                                             bias=bv_sb[mt][:], scale=1.0)
                        for ss in range(4):
                            k = st * 4 + ss
                            pt = pst.tile([128, 128], f16, tag="pt")
                            nc.tensor.transpose(pt[:], vt[:, ss * 128:(ss + 1) * 128],
                                                ident[:])
                            src = pt[:].rearrange("p (h x) -> p h x", h=2)
                            dst = v_ext[k][:].rearrange("p (h x) -> p h x", h=4)
                            nc.vector.tensor_copy(dst[:, 2 * mt:2 * mt + 2, 0:64], src)
                for k in range(SB):
                    pad = v_ext[k][:].rearrange("p (h x) -> p h x", h=4)[:, :, 64:128]
                    nc.gpsimd.memset(pad, 0.0)
                    one_col = v_ext[k][:].rearrange("p (h x) -> p h x", h=4)[:, :, 64:65]
                    nc.gpsimd.memset(one_col, 1.0)

            # ---------------- phase 3: attention ----------------
            with (
                tc.tile_pool(name="exs", bufs=5) as pex,
                tc.tile_pool(name="recs", bufs=2) as prec,
                tc.tile_pool(name="psim", bufs=3, space="PSUM") as psim,
                tc.tile_pool(name="pspv", bufs=1, space="PSUM") as pspv,
                tc.tile_pool(name="psbc", bufs=1, space="PSUM") as psbc,
            ):
                def head_work(g):
                    h = g // G
                    ti, slot = _qslot(g)
                    qg = qp_sb[ti][slot * 64:(slot + 1) * 64, :]
                    kh = kT_sb[h // 2][(h % 2) * 64:(h % 2) * 64 + 64, :]
                    pv = pspv.tile([128, 512], f32, tag="pv", name="pv")
                    exs = [None] * 6

                    def do_sim(t):
                        st_ = psim.tile([128, TILE_W[t]], f32, tag="sim", name="sim")
                        for k in TILE_BLOCKS[t]:
                            _, off, n0, w = BLK[k]
                            nc.tensor.matmul(
                                st_[:, off:off + w],
                                kh[:, k * 128:(k + 1) * 128], qg[:, n0:512],
                                start=True, stop=True)
                        ex = pex.tile([128, TILE_W[t]], f16, tag="ex", name="ex")
                        exs[t] = ex
                        for lo, hi in SPANS[t]:
                            nc.scalar.activation(ex[:, lo:hi], st_[:, lo:hi], AF.Exp)
                            nc.vector.tensor_tensor(
                                ex[:, lo:hi], ex[:, lo:hi], mask_sb[t][:, lo:hi],
                                op=OP.mult)

                    def do_pv(t):
                        ex = exs[t]
                        for k in TILE_BLOCKS[t]:
                            _, off, n0, w = BLK[k]
                            nc.tensor.matmul(
                                pv[:, n0:512], v_ext[k][:, 128 * h:128 * h + 128],
                                ex[:, off:off + w],
                                start=(k == 0), stop=(k == SB - 1),
                                skip_group_check=True)

                    def den_copy():
                        # den row (partition 64) to SBUF right after PV ends
                        den = prec.tile([65, 512], f32r, tag="den", name="den")
                        nc.vector.tensor_copy(den[64:65, :], pv[64:65, :])
                        return den

                    def tail(den):
                        # issued one head later so the PE's bc matmul never waits
                        bc = psbc.tile([64, 512], f32, tag="bc", name="bc")
                        nc.tensor.matmul(bc[:], ones64r[64:65, :], den[64:65, :],
                                         start=True, stop=True)
                        rec = prec.tile([64, 512], f32, tag="rec", name="rec")
                        nc.vector.reciprocal_approx_fast(rec[:], bc[:])
                        p0 = (g % 2) * 64
                        att = attT[g // 2][p0:p0 + 64, :]
                        nc.vector.tensor_tensor(att, pv[0:64, :], rec[:], op=OP.mult)

                    return do_sim, do_pv, den_copy, tail

                prev_tail = None
                for g in range(HQ):
                    do_sim, do_pv, den_copy, tail = head_work(g)
                    # software pipeline: sim runs 2-3 tiles ahead of pv;
                    # previous head's tail issues after this head's first sims.
                    do_sim(0)
                    do_sim(1)
                    if prev_tail is not None:
                        prev_tail()
                    do_sim(2)
                    do_pv(0)
                    do_sim(3)
                    do_pv(1)
                    do_sim(4)
                    do_pv(2)
                    do_sim(5)
                    do_pv(3)
                    do_pv(4)
                    do_pv(5)
                    den = den_copy()
                    prev_tail = (lambda t=tail, d=den: t(d))
                prev_tail()

            # ---------------- phase 4: layernorm + out projection ----------------
            with (
                tc.tile_pool(name="lnt", bufs=2) as plnt,
                tc.tile_pool(name="ysb", bufs=2) as pysb,
                tc.tile_pool(name="psst", bufs=1, space="PSUM") as psst,
                tc.tile_pool(name="pslb", bufs=1, space="PSUM") as pslb,
                tc.tile_pool(name="psy", bufs=2, space="PSUM") as psy,
            ):
                st_sum = psst.tile([1, NL], f32, tag="ssum")
                st_sq = psst.tile([1, NL], f32, tag="ssq")
                for e in range(EC):
                    nc.tensor.matmul(st_sum[:], ones_m1[:], attT[e][:],
                                     start=(e == 0), stop=(e == EC - 1))
                    sq = plnt.tile([128, NL], f16, tag="sq")
                    nc.scalar.activation(sq[:], attT[e][:], AF.Square)
                    nc.tensor.matmul(st_sq[:], ones_m1[:], sq[:],
                                     start=(e == 0), stop=(e == EC - 1))
                mu = plnt.tile([1, NL], f32r, tag="mu")
                nc.vector.tensor_scalar_mul(mu[:], st_sum[:], 1.0 / E)
                var = plnt.tile([1, NL], f32, tag="var")
                nc.vector.tensor_scalar_mul(var[:], st_sq[:], 1.0 / E)
                mu2 = plnt.tile([1, NL], f32, tag="mu2")
                nc.vector.tensor_tensor(mu2[:], mu[:], mu[:], op=OP.mult)
                nc.vector.tensor_tensor(var[:], var[:], mu2[:], op=OP.subtract)
                sd = plnt.tile([1, NL], f32, tag="sd")
                nc.scalar.activation(sd[:], var[:], AF.Sqrt, bias=eps_c[:])
                nc.vector.reciprocal_approx_fast(sd[:], sd[:])
                sdr = plnt.tile([1, NL], f32r, tag="sdr")
                nc.vector.tensor_copy(sdr[:], sd[:])
                mb = pslb.tile([128, NL], f32, tag="mb")
                nc.tensor.matmul(mb[:], ones_k1[:], mu[:], start=True, stop=True)
                ib = pslb.tile([128, NL], f32, tag="ib")
                nc.tensor.matmul(ib[:], ones_k1[:], sdr[:], start=True, stop=True)
                for e in range(EC):
                    tmp = plnt.tile([128, NL], f32, tag="xn")
                    nc.vector.tensor_tensor(tmp[:], attT[e][:], mb[:], op=OP.subtract)
                    nc.vector.tensor_tensor(tmp[:], tmp[:], ib[:], op=OP.mult)
                    nc.vector.tensor_scalar(attT[e][:], tmp[:], lng_sb[e][:],
                                            lnb_sb[e][:], op0=OP.mult, op1=OP.add)
                for nt in range(4):
                    pyA = psy.tile([128, 512], f32, tag="pyA", name="pyA")
                    pyB = psy.tile([128, 512], f32, tag="pyB", name="pyB")
                    pys = (pyA, pyB)
                    for e in range(EC):
                        for oc in range(2):
                            nc.tensor.matmul(
                                pys[oc][:], attT[e][:, nt * 128:(nt + 1) * 128],
                                wo_sb[e][:, oc * 512:(oc + 1) * 512],
                                start=(e == 0), stop=False)
                    for oc in range(2):
                        nc.tensor.matmul(pys[oc][:], ones_k1h[:],
                                         bo_sb[0:1, oc * 512:(oc + 1) * 512],
                                         start=False, stop=True)
                        ys = pysb.tile([128, 512], f32, tag="ys")
                        nc.vector.tensor_copy(ys[:], pys[oc][:])
                        nc.sync.dma_start(
                            y[nt * 128:(nt + 1) * 128, oc * 512:(oc + 1) * 512],
                            ys[:])
    nc.finalize()
    return nc


def _build_masks(adjc):
    """adjc: [NL, N] int (causal&adj premasked). Returns 6 f16 tiles in the
    block layout (tile cols <-> (block, n-range))."""
    at = adjc.T.astype(np.float16)  # [N, NL] = [s, n]
    tiles = [np.zeros((128, TILE_W[t]), np.float16) for t in range(6)]
    for k in range(SB):
        t, off, n0, w = BLK[k]
        tiles[t][:, off:off + w] = at[k * 128:(k + 1) * 128, n0:512]
    return tiles


def shard_inputs(inputs):
    q = np.asarray(inputs["query"], np.float32)
    k = np.asarray(inputs["key"], np.float32)
    v = np.asarray(inputs["value"], np.float32)
    adj = np.asarray(inputs["adj"], np.int32)
    WqT8 = (np.ascontiguousarray(np.asarray(inputs["Wq"], np.float32).T)
            / np.float32(8.0)).astype(np.float16)
    WkT = np.ascontiguousarray(np.asarray(inputs["Wk"], np.float32).T).astype(np.float16)
    WvT = np.ascontiguousarray(np.asarray(inputs["Wv"], np.float32).T).astype(np.float16)
    WoT = np.ascontiguousarray(np.asarray(inputs["Wo"], np.float32).T).astype(np.float16)
    bq8 = (np.asarray(inputs["bq"], np.float32) / np.float32(8.0)).reshape(EC, 128)
    bk2 = np.asarray(inputs["bk"], np.float32).reshape(2, 128)
    bv2 = np.asarray(inputs["bv"], np.float32).reshape(2, 128)
    bo1 = np.asarray(inputs["bo"], np.float32).reshape(1, E).astype(np.float16)
    lng = np.asarray(inputs["ln_g"], np.float32).reshape(EC, 128)
    lnb = np.asarray(inputs["ln_b"], np.float32).reshape(EC, 128)

    shared = dict(WqT=WqT8, WkT=WkT, WvT=WvT, WoT=WoT, bq2d=bq8, bk2d=bk2,
                  bv2d=bv2, bo1=bo1, lng=lng, lnb=lnb,
                  ones1=np.ones((1, 128), np.float32),
                  ones1h=np.ones((1, 128), np.float16))
    per_b = []
    s_idx = np.arange(N)
    for b in range(B):
        per_b.append((np.ascontiguousarray(k[b].T).astype(np.float16),
                      np.ascontiguousarray(v[b].T).astype(np.float16)))
    in_maps = []
    for c in range(8):
        b, j = divmod(c, 4)
        rows = np.arange(j, N, 4)
        causal = s_idx[None, :] <= rows[:, None]          # [NL, N]
        adjc = np.where(causal, adj[b][rows], 0)
        m = dict(shared)
        m["xqT"] = np.ascontiguousarray(q[b][rows].T).astype(np.float16)
        m["xkT"], m["xvT"] = per_b[b]
        for t, mk in enumerate(_build_masks(adjc)):
            m[f"mask{t}"] = mk
        in_maps.append(m)
    return in_maps


def _host_fixup(out, inputs):
    """Rows with no unmasked causal position get the reference's uniform-
    softmax-over-everything fallback, computed exactly on host."""
    adj = np.asarray(inputs["adj"])
    s_idx = np.arange(N)
    causal = s_idx[None, :] <= s_idx[:, None]
    for b in range(B):
        amr = np.where(((adj[b] != 0) & causal).sum(1) == 0)[0]
        if len(amr) == 0:
            continue
        v = np.asarray(inputs["value"][b], np.float64)
        Wv = np.asarray(inputs["Wv"], np.float64)
        bv = np.asarray(inputs["bv"], np.float64)
        vp = v @ Wv.T + bv                       # [N, KVE]
        mv = vp.mean(0)                          # [KVE]
        row = np.concatenate([mv[(k // G) * D:(k // G) * D + D] for k in range(HQ)])
        mu = row.mean()
        var = ((row - mu) ** 2).mean()
        rown = (row - mu) / np.sqrt(var + LN_EPS)
        rown = rown * np.asarray(inputs["ln_g"], np.float64) + np.asarray(
            inputs["ln_b"], np.float64)
        yrow = rown @ np.asarray(inputs["Wo"], np.float64).T + np.asarray(
            inputs["bo"], np.float64)
        out[b, amr, :] = yrow.astype(np.float32)
    return out


def unshard_outputs(results, inputs):
    out = np.empty((B, N, E), np.float32)
    for c in range(8):
        b, j = divmod(c, 4)
        out[b, j::4, :] = results[c]["y"]
    return _host_fixup(out, inputs)


def kernel(**inputs):
    from concourse.bass_utils import run_bass_kernel_spmd

    if "nc" not in _PROG_CACHE:
        _PROG_CACHE["nc"] = build_program()
    nc = _PROG_CACHE["nc"]
    in_maps = shard_inputs(inputs)
    res = run_bass_kernel_spmd(nc, in_maps, core_ids=list(range(8)))
    return unshard_outputs(res.results, inputs)


# revision 3
# speedup vs baseline: 1.0448x; 1.0448x over previous
"""GQA sparse-attention kernel for 8 Trainium2 NeuronCores.

Sharding: data-parallel over batch (2) x sequence-parallel over query rows
(rows j::4 interleaved, so causal work is balanced and the program is
SPMD-identical across cores). No collectives. Each core: q-proj for its 512
rows, full k/v proj for its batch, 16-head attention, layernorm + out-proj.

Key performance structure (vs the naive phase pipeline):
  - exact causal trimming of sim/PV/exp/mask work: per s-block k (128 wide),
    only query columns n >= 32k are computed (local row n <-> global 4n+j).
    Blocks are packed into PSUM tiles at bank-aligned offsets so exp/mask run
    on a few contiguous spans. sim+PV columns drop from 6144 to 4352 per head.
  - all-masked-row fallback moved to host (tiny numpy fixup on exact rows);
    the device lets those rows go NaN and the host overwrites them.
  - all input DMAs issued upfront as few big transfers (SP queue for the
    p1/p2 critical path, ACT queue for masks/small consts/p4 weights); adj
    arrives as prebuilt f16 mask tiles in the device block layout.
  - phase 3 software pipeline tuned to keep the PE instruction queue from
    ever stalling: the TRN2 tensor engine's clock drops to a mid p-state on
    any queue stall and needs ~3us of continuous execution to re-reach
    2.4GHz, so sim matmuls run 2-3 PSUM tiles ahead of the dependent PV
    matmuls (psim bufs=3) and each head's denominator tail (PE broadcast +
    DVE reciprocal + multiply) is issued one head late so its semaphore wait
    is pre-satisfied.
"""

import os
import sys

import numpy as np

for _p in ("/opt/trn_rl_repo", "/root/.axon_site/_ro/trn_rl_repo"):
    if os.path.isdir(_p) and _p not in sys.path:
        sys.path.insert(0, _p)

B, N, E = 2, 2048, 1024
HQ, HK, D = 16, 4, 64
G = HQ // HK          # 4 query heads per kv head
KVE = HK * D          # 256
NL = N // 4           # 512 local query rows per core
SB = N // 128         # 16 s-blocks
EC = E // 128         # 8 embedding chunks
LN_EPS = 1e-5

# s-block layout: block k -> (tile index, col offset, n0, width)
# tile A..E are [128,1024] (2 PSUM banks), F is [128,512] (1 bank)
BLK = [
    (0, 0, 0, 512), (0, 512, 32, 480),
    (1, 0, 64, 448), (1, 512, 96, 416),
    (2, 0, 128, 384), (2, 512, 160, 352),
    (3, 0, 192, 320), (3, 512, 224, 288),
    (4, 0, 256, 256), (4, 256, 288, 224), (4, 512, 320, 192), (4, 704, 352, 160),
    (5, 0, 384, 128), (5, 128, 416, 96), (5, 224, 448, 64), (5, 288, 480, 32),
]
# contiguous exp/mask spans per tile: (col_lo, col_hi)
SPANS = [
    [(0, 992)],
    [(0, 448), (512, 928)],
    [(0, 384), (512, 864)],
    [(0, 320), (512, 800)],
    [(0, 480), (512, 864)],
    [(0, 320)],
]
TILE_W = [1024, 1024, 1024, 1024, 1024, 512]
TILE_BLOCKS = [[0, 1], [2, 3], [4, 5], [6, 7], [8, 9, 10, 11], [12, 13, 14, 15]]

_PROG_CACHE = {}


def build_program():
    import concourse.mybir as mybir
    import concourse.tile as tile
    from concourse import bacc

    dt = mybir.dt
    f32, f32r, f16, i32 = dt.float32, dt.float32r, dt.float16, dt.int32
    AF = mybir.ActivationFunctionType
    OP = mybir.AluOpType
    AX = mybir.AxisListType

    nc = bacc.Bacc("TRN2", target_bir_lowering=False, debug=False)

    def din(name, shape, dtp=f32):
        return nc.dram_tensor(name, shape, dtp, kind="ExternalInput").ap()

    xqT = din("xqT", [E, NL], f16)
    xkT = din("xkT", [E, N], f16)
    xvT = din("xvT", [E, N], f16)
    WqT = din("WqT", [E, E], f16)          # pre-scaled by 1/8 on host
    WkT = din("WkT", [E, KVE], f16)
    WvT = din("WvT", [E, KVE], f16)
    WoT = din("WoT", [E, E], f16)
    bq2d = din("bq2d", [EC, 128])          # bq/8
    bk2d = din("bk2d", [2, 128])
    bv2d = din("bv2d", [2, 128])
    bo1 = din("bo1", [1, E], f16)
    lng = din("lng", [EC, 128])
    lnb = din("lnb", [EC, 128])
    ones1 = din("ones1", [1, 128], f32r)
    ones1h = din("ones1h", [1, 128], f16)
    masks_in = [din(f"mask{t}", [128, TILE_W[t]], f16) for t in range(6)]
    y = nc.dram_tensor("y", [NL, E], f32, kind="ExternalOutput").ap()

    with tile.TileContext(nc) as tc, nc.allow_low_precision(
            "f16/f32r operands for PE fast-path matmuls are intentional"):
        with (
            tc.tile_pool(name="const", bufs=1) as pc,
            tc.tile_pool(name="persist", bufs=1) as pp,
            tc.tile_pool(name="bigx", bufs=1) as pbx,
        ):
            # ---- upfront DMA prefetch ----
            # SP queue: p1 then p2 critical path (few big DMAs)
            wq_all = pc.tile([128, EC * E], f16, tag="wq_all")
            wq_sb = [wq_all[:, e * E:(e + 1) * E] for e in range(EC)]
            nc.sync.dma_start(
                wq_all[:].rearrange("p (e c) -> p e c", e=EC),
                WqT.rearrange("(e p) c -> p e c", e=EC))
            xq_all = pc.tile([128, EC * NL], f16, tag="xq_all")
            xq_sb = [xq_all[:, e * NL:(e + 1) * NL] for e in range(EC)]
            nc.sync.dma_start(
                xq_all[:].rearrange("p (e c) -> p e c", e=EC),
                xqT.rearrange("(e p) c -> p e c", e=EC))
            xk_all = pbx.tile([128, EC * N], f16, tag="xk_all")
            xv_all = pbx.tile([128, EC * N], f16, tag="xv_all")
            xk_sb = [xk_all[:, e * N:(e + 1) * N] for e in range(EC)]
            xv_sb = [xv_all[:, e * N:(e + 1) * N] for e in range(EC)]
            for st in range(4):
                ssl = slice(st * 512, (st + 1) * 512)
                nc.sync.dma_start(
                    xk_all[:].rearrange("p (e c) -> p e c", e=EC)[:, :, ssl],
                    xkT.rearrange("(e p) c -> p e c", e=EC)[:, :, ssl])
                nc.sync.dma_start(
                    xv_all[:].rearrange("p (e c) -> p e c", e=EC)[:, :, ssl],
                    xvT.rearrange("(e p) c -> p e c", e=EC)[:, :, ssl])
            # ACT queue: k/v weights, small consts, masks, p4 weights
            wk_all = pc.tile([128, EC * KVE], f16, tag="wk_all")
            wv_all = pc.tile([128, EC * KVE], f16, tag="wv_all")
            wk_sb = [wk_all[:, e * KVE:(e + 1) * KVE] for e in range(EC)]
            wv_sb = [wv_all[:, e * KVE:(e + 1) * KVE] for e in range(EC)]
            nc.scalar.dma_start(
                wk_all[:].rearrange("p (e c) -> p e c", e=EC),
                WkT.rearrange("(e p) c -> p e c", e=EC))
            nc.scalar.dma_start(
                wv_all[:].rearrange("p (e c) -> p e c", e=EC),
                WvT.rearrange("(e p) c -> p e c", e=EC))

            mask_sb = [pp.tile([128, TILE_W[t]], f16, tag=f"mk{t}", name=f"mk{t}")
                       for t in range(6)]
            for t in range(6):
                nc.scalar.dma_start(mask_sb[t][:], masks_in[t])
            bq_sb = [pc.tile([128, 1], f32, tag=f"bq{m}", name=f"bq{m}") for m in range(EC)]
            for e in range(EC):
                nc.scalar.dma_start(bq_sb[e][:], bq2d[e:e + 1, :])
            bk_sb = [pc.tile([128, 1], f32, tag=f"bk{m}", name=f"bk{m}") for m in range(2)]
            bv_sb = [pc.tile([128, 1], f32, tag=f"bv{m}", name=f"bv{m}") for m in range(2)]
            for m in range(2):
                nc.scalar.dma_start(bk_sb[m][:], bk2d[m:m + 1, :])
                nc.scalar.dma_start(bv_sb[m][:], bv2d[m:m + 1, :])
            ones_k1 = pc.tile([1, 128], f32r, tag="ones_k1")
            nc.scalar.dma_start(ones_k1[:], ones1)
            ones_m1 = pc.tile([128, 1], f16, tag="ones_m1")
            nc.scalar.dma_start(ones_m1[:], ones1h)
            ones_k1h = pc.tile([1, 128], f16, tag="ones_k1h")
            nc.scalar.dma_start(ones_k1h[:], ones1h)
            lng_sb = [pp.tile([128, 1], f32, tag=f"lng{e}", name=f"lng{e}") for e in range(EC)]
            lnb_sb = [pp.tile([128, 1], f32, tag=f"lnb{e}", name=f"lnb{e}") for e in range(EC)]
            for e in range(EC):
                nc.scalar.dma_start(lng_sb[e][:], lng[e:e + 1, :])
                nc.scalar.dma_start(lnb_sb[e][:], lnb[e:e + 1, :])
            bo_sb = pp.tile([1, E], f16, tag="bo", name="bo")
            nc.scalar.dma_start(bo_sb[:], bo1)
            wo_all = pp.tile([128, EC * E], f16, tag="wo_all", name="wo_all")
            wo_sb = [wo_all[:, e * E:(e + 1) * E] for e in range(EC)]
            nc.scalar.dma_start(
                wo_all[:].rearrange("p (e c) -> p e c", e=EC),
                WoT.rearrange("(e p) c -> p e c", e=EC))

            ident = pc.tile([128, 128], f16, tag="ident")
            from concourse.masks import make_identity
            make_identity(nc, ident[:])
            eps_c = pc.tile([1, 1], f32, tag="eps_c")
            nc.gpsimd.memset(eps_c[:], LN_EPS)
            ones64r = pc.tile([65, 64], f32r, tag="ones64r")
            nc.scalar.dma_start(ones64r[64:65, :], ones1[:, 0:64])

            # persistent activation tiles
            kT_sb = [pp.tile([128, N], f16, tag=f"kt{m}", name=f"kt{m}") for m in range(2)]
            v_ext = [pp.tile([128, 4 * 128], f16, tag=f"vx{k}", name=f"vx{k}") for k in range(SB)]
            qp_sb = [pp.tile([128, NL], f16, tag=f"qp{m}", name=f"qp{m}") for m in range(EC)]
            attT = [pp.tile([128, NL], f16, tag=f"at{e}", name=f"at{e}") for e in range(EC)]
            _EVEN = [0, 1, 2, 3, 8, 9, 10, 11]    # heads whose kv head is even
            _ODD = [4, 5, 6, 7, 12, 13, 14, 15]

            def _qslot(g):
                if (g // G) % 2 == 0:
                    return _EVEN.index(g), 0
                return _ODD.index(g), 1

            # ---------------- phase 1: q projection ----------------
            with tc.tile_pool(name="psq", bufs=2, space="PSUM") as psq:
                for mt2 in range(EC // 2):
                    psA = psq.tile([128, NL], f32, tag="psqA", name="psqA")
                    psB = psq.tile([128, NL], f32, tag="psqB", name="psqB")
                    for e in range(EC):
                        for mt, ps in ((2 * mt2, psA), (2 * mt2 + 1, psB)):
                            nc.tensor.matmul(
                                ps[:], wq_sb[e][:, mt * 128:(mt + 1) * 128],
                                xq_sb[e][:], start=(e == 0), stop=(e == EC - 1))
                    for mt, ps in ((2 * mt2, psA), (2 * mt2 + 1, psB)):
                        for t in range(2):
                            g = 2 * mt + t
                            ti, slot = _qslot(g)
                            nc.scalar.activation(
                                qp_sb[ti][slot * 64:(slot + 1) * 64, :],
                                ps[t * 64:(t + 1) * 64, :], AF.Identity,
                                bias=bq_sb[mt][t * 64:(t + 1) * 64, :], scale=1.0)

            # ---------------- phase 2: k/v projections ----------------
            with (
                tc.tile_pool(name="vt", bufs=2) as pvt,
                tc.tile_pool(name="pskv", bufs=2, space="PSUM") as pskv,
                tc.tile_pool(name="pst", bufs=2, space="PSUM") as pst,
            ):
                for st in range(4):  # s-tiles of 512
                    sl = slice(st * 512, (st + 1) * 512)
                    for mt in range(2):
                        psk = pskv.tile([128, 512], f32, tag="psk")
                        psv = pskv.tile([128, 512], f32, tag="psv")
                        for e in range(EC):
                            nc.tensor.matmul(
                                psk[:], wk_sb[e][:, mt * 128:(mt + 1) * 128],
                                xk_sb[e][:, sl], start=(e == 0), stop=(e == EC - 1))
                            nc.tensor.matmul(
                                psv[:], wv_sb[e][:, mt * 128:(mt + 1) * 128],
                                xv_sb[e][:, sl], start=(e == 0), stop=(e == EC - 1))
                        nc.scalar.activation(kT_sb[mt][:, sl], psk[:], AF.Identity,
                                             bias=bk_sb[mt][:], scale=1.0)
                        vt = pvt.tile([128, 512], f16, tag="vt")
                        nc.scalar.activation(vt[:], psv[:], AF.Identity,
                                             bias=bv_sb[mt][:], scale=1.0)
                        for ss in range(4):
                            k = st * 4 + ss
                            pt = pst.tile([128, 128], f16, tag="pt")
                            nc.tensor.transpose(pt[:], vt[:, ss * 128:(ss + 1) * 128],
                                                ident[:])
                            src = pt[:].rearrange("p (h x) -> p h x", h=2)
                            dst = v_ext[k][:].rearrange("p (h x) -> p h x", h=4)
                            nc.vector.tensor_copy(dst[:, 2 * mt:2 * mt + 2, 0:64], src)
                for k in range(SB):
                    pad = v_ext[k][:].rearrange("p (h x) -> p h x", h=4)[:, :, 64:128]
                    nc.gpsimd.memset(pad, 0.0)
                    one_col = v_ext[k][:].rearrange("p (h x) -> p h x", h=4)[:, :, 64:65]
                    nc.gpsimd.memset(one_col, 1.0)

            # ---------------- phase 3: attention ----------------
            with (
                tc.tile_pool(name="exs", bufs=5) as pex,
                tc.tile_pool(name="recs", bufs=2) as prec,
                tc.tile_pool(name="psim", bufs=3, space="PSUM") as psim,
                tc.tile_pool(name="pspv", bufs=1, space="PSUM") as pspv,
                tc.tile_pool(name="psbc", bufs=1, space="PSUM") as psbc,
            ):
                def head_work(g):
                    h = g // G
                    ti, slot = _qslot(g)
                    qg = qp_sb[ti][slot * 64:(slot + 1) * 64, :]
                    kh = kT_sb[h // 2][(h % 2) * 64:(h % 2) * 64 + 64, :]
                    pv = pspv.tile([128, 512], f32, tag="pv", name="pv")
                    exs = [None] * 6

                    def do_sim(t):
                        st_ = psim.tile([128, TILE_W[t]], f32, tag="sim", name="sim")
                        for k in TILE_BLOCKS[t]:
                            _, off, n0, w = BLK[k]
                            nc.tensor.matmul(
                                st_[:, off:off + w],
                                kh[:, k * 128:(k + 1) * 128], qg[:, n0:512],
                                start=True, stop=True)
                        ex = pex.tile([128, TILE_W[t]], f16, tag="ex", name="ex")
                        exs[t] = ex
                        for lo, hi in SPANS[t]:
                            nc.scalar.activation(ex[:, lo:hi], st_[:, lo:hi], AF.Exp)
                            nc.vector.tensor_tensor(
                                ex[:, lo:hi], ex[:, lo:hi], mask_sb[t][:, lo:hi],
                                op=OP.mult)

                    def do_pv(t):
                        ex = exs[t]
                        for k in TILE_BLOCKS[t]:
                            _, off, n0, w = BLK[k]
                            nc.tensor.matmul(
                                pv[:, n0:512], v_ext[k][:, 128 * h:128 * h + 128],
                                ex[:, off:off + w],
                                start=(k == 0), stop=(k == SB - 1),
                                skip_group_check=True)

                    def den_copy():
                        # den row (partition 64) to SBUF right after PV ends
                        den = prec.tile([65, 512], f32r, tag="den", name="den")
                        nc.vector.tensor_copy(den[64:65, :], pv[64:65, :])
                        return den

                    def tail(den):
                        # issued one head later so the PE's bc matmul never waits
                        bc = psbc.tile([64, 512], f32, tag="bc", name="bc")
                        nc.tensor.matmul(bc[:], ones64r[64:65, :], den[64:65, :],
                                         start=True, stop=True)
                        rec = prec.tile([64, 512], f32, tag="rec", name="rec")
                        nc.vector.reciprocal_approx_fast(rec[:], bc[:])
                        p0 = (g % 2) * 64
                        att = attT[g // 2][p0:p0 + 64, :]
                        nc.vector.tensor_tensor(att, pv[0:64, :], rec[:], op=OP.mult)

                    return do_sim, do_pv, den_copy, tail

                prev_tail = None
                for g in range(HQ):
                    do_sim, do_pv, den_copy, tail = head_work(g)
                    # software pipeline: sim runs 2-3 tiles ahead of pv;
                    # previous head's tail issues after this head's first sims.
                    do_sim(0)
                    do_sim(1)
                    if prev_tail is not None:
                        prev_tail()
                    do_sim(2)
                    do_pv(0)
                    do_sim(3)
                    do_pv(1)
                    do_sim(4)
                    do_pv(2)
                    do_sim(5)
                    do_pv(3)
                    do_pv(4)
                    do_pv(5)
                    den = den_copy()
                    prev_tail = (lambda t=tail, d=den: t(d))
                prev_tail()

            # ---------------- phase 4: layernorm + out projection ----------------
            with (
                tc.tile_pool(name="lnt", bufs=2) as plnt,
                tc.tile_pool(name="ysb", bufs=2) as pysb,
                tc.tile_pool(name="psst", bufs=1, space="PSUM") as psst,
                tc.tile_pool(name="pslb", bufs=1, space="PSUM") as pslb,
                tc.tile_pool(name="psy", bufs=2, space="PSUM") as psy,
            ):
                st_sum = psst.tile([1, NL], f32, tag="ssum")
                st_sq = psst.tile([1, NL], f32, tag="ssq")
                for e in range(EC):
                    nc.tensor.matmul(st_sum[:], ones_m1[:], attT[e][:],
                                     start=(e == 0), stop=(e == EC - 1))
                    sq = plnt.tile([128, NL], f16, tag="sq")
                    nc.scalar.activation(sq[:], attT[e][:], AF.Square)
                    nc.tensor.matmul(st_sq[:], ones_m1[:], sq[:],
                                     start=(e == 0), stop=(e == EC - 1))
                mu = plnt.tile([1, NL], f32r, tag="mu")
                nc.vector.tensor_scalar_mul(mu[:], st_sum[:], 1.0 / E)
                var = plnt.tile([1, NL], f32, tag="var")
                nc.vector.tensor_scalar_mul(var[:], st_sq[:], 1.0 / E)
                mu2 = plnt.tile([1, NL], f32, tag="mu2")
                nc.vector.tensor_tensor(mu2[:], mu[:], mu[:], op=OP.mult)
                nc.vector.tensor_tensor(var[:], var[:], mu2[:], op=OP.subtract)
                sd = plnt.tile([1, NL], f32, tag="sd")
                nc.scalar.activation(sd[:], var[:], AF.Sqrt, bias=eps_c[:])
                nc.vector.reciprocal_approx_fast(sd[:], sd[:])
                sdr = plnt.tile([1, NL], f32r, tag="sdr")
                nc.vector.tensor_copy(sdr[:], sd[:])
                mb = pslb.tile([128, NL], f32, tag="mb")
                nc.tensor.matmul(mb[:], ones_k1[:], mu[:], start=True, stop=True)
                ib = pslb.tile([128, NL], f32, tag="ib")
                nc.tensor.matmul(ib[:], ones_k1[:], sdr[:], start=True, stop=True)
                for e in range(EC):
                    tmp = plnt.tile([128, NL], f32, tag="xn")
                    nc.vector.tensor_tensor(tmp[:], attT[e][:], mb[:], op=OP.subtract)
                    nc.vector.tensor_tensor(tmp[:], tmp[:], ib[:], op=OP.mult)
                    nc.vector.tensor_scalar(attT[e][:], tmp[:], lng_sb[e][:],
                                            lnb_sb[e][:], op0=OP.mult, op1=OP.add)
                for nt in range(4):
                    pyA = psy.tile([128, 512], f32, tag="pyA", name="pyA")
                    pyB = psy.tile([128, 512], f32, tag="pyB", name="pyB")
                    pys = (pyA, pyB)
                    for e in range(EC):
                        for oc in range(2):
                            nc.tensor.matmul(
                                pys[oc][:], attT[e][:, nt * 128:(nt + 1) * 128],
                                wo_sb[e][:, oc * 512:(oc + 1) * 512],
                                start=(e == 0), stop=False)
                    for oc in range(2):
                        nc.tensor.matmul(pys[oc][:], ones_k1h[:],
                                         bo_sb[0:1, oc * 512:(oc + 1) * 512],
                                         start=False, stop=True)
                        ys = pysb.tile([128, 512], f32, tag="ys")
                        nc.vector.tensor_copy(ys[:], pys[oc][:])
                        nc.sync.dma_start(
                            y[nt * 128:(nt + 1) * 128, oc * 512:(oc + 1) * 512],
                            ys[:])
    nc.finalize()
    return nc


def _build_masks(adjc):
    """adjc: [NL, N] int (causal&adj premasked). Returns 6 f16 tiles in the
    block layout (tile cols <-> (block, n-range))."""
    at = adjc.T.astype(np.float16)  # [N, NL] = [s, n]
    tiles = [np.zeros((128, TILE_W[t]), np.float16) for t in range(6)]
    for k in range(SB):
        t, off, n0, w = BLK[k]
        tiles[t][:, off:off + w] = at[k * 128:(k + 1) * 128, n0:512]
    return tiles


def shard_inputs(inputs):
    q = np.asarray(inputs["query"], np.float32)
    k = np.asarray(inputs["key"], np.float32)
    v = np.asarray(inputs["value"], np.float32)
    adj = np.asarray(inputs["adj"], np.int32)
    WqT8 = (np.ascontiguousarray(np.asarray(inputs["Wq"], np.float32).T)
            / np.float32(8.0)).astype(np.float16)
    WkT = np.ascontiguousarray(np.asarray(inputs["Wk"], np.float32).T).astype(np.float16)
    WvT = np.ascontiguousarray(np.asarray(inputs["Wv"], np.float32).T).astype(np.float16)
    WoT = np.ascontiguousarray(np.asarray(inputs["Wo"], np.float32).T).astype(np.float16)
    bq8 = (np.asarray(inputs["bq"], np.float32) / np.float32(8.0)).reshape(EC, 128)
    bk2 = np.asarray(inputs["bk"], np.float32).reshape(2, 128)
    bv2 = np.asarray(inputs["bv"], np.float32).reshape(2, 128)
    bo1 = np.asarray(inputs["bo"], np.float32).reshape(1, E).astype(np.float16)
    lng = np.asarray(inputs["ln_g"], np.float32).reshape(EC, 128)
    lnb = np.asarray(inputs["ln_b"], np.float32).reshape(EC, 128)

    shared = dict(WqT=WqT8, WkT=WkT, WvT=WvT, WoT=WoT, bq2d=bq8, bk2d=bk2,
                  bv2d=bv2, bo1=bo1, lng=lng, lnb=lnb,
                  ones1=np.ones((1, 128), np.float32),
                  ones1h=np.ones((1, 128), np.float16))
    per_b = []
    s_idx = np.arange(N)
    for b in range(B):
        per_b.append((np.ascontiguousarray(k[b].T).astype(np.float16),
                      np.ascontiguousarray(v[b].T).astype(np.float16)))
    in_maps = []
    for c in range(8):
        b, j = divmod(c, 4)
        rows = np.arange(j, N, 4)
        causal = s_idx[None, :] <= rows[:, None]          # [NL, N]
        adjc = np.where(causal, adj[b][rows], 0)
        m = dict(shared)
        m["xqT"] = np.ascontiguousarray(q[b][rows].T).astype(np.float16)
        m["xkT"], m["xvT"] = per_b[b]
        for t, mk in enumerate(_build_masks(adjc)):
            m[f"mask{t}"] = mk
        in_maps.append(m)
    return in_maps


def _host_fixup(out, inputs):
    """Rows with no unmasked causal position get the reference's uniform-
    softmax-over-everything fallback, computed exactly on host."""
    adj = np.asarray(inputs["adj"])
    s_idx = np.arange(N)
    causal = s_idx[None, :] <= s_idx[:, None]
    for b in range(B):
        amr = np.where(((adj[b] != 0) & causal).sum(1) == 0)[0]
        if len(amr) == 0:
            continue
        v = np.asarray(inputs["value"][b], np.float64)
        Wv = np.asarray(inputs["Wv"], np.float64)
        bv = np.asarray(inputs["bv"], np.float64)
        vp = v @ Wv.T + bv                       # [N, KVE]
        mv = vp.mean(0)                          # [KVE]
        row = np.concatenate([mv[(k // G) * D:(k // G) * D + D] for k in range(HQ)])
        mu = row.mean()
        var = ((row - mu) ** 2).mean()
        rown = (row - mu) / np.sqrt(var + LN_EPS)
        rown = rown * np.asarray(inputs["ln_g"], np.float64) + np.asarray(
            inputs["ln_b"], np.float64)
        yrow = rown @ np.asarray(inputs["Wo"], np.float64).T + np.asarray(
            inputs["bo"], np.float64)
        out[b, amr, :] = yrow.astype(np.float32)
    return out


def unshard_outputs(results, inputs):
    out = np.empty((B, N, E), np.float32)
    for c in range(8):
        b, j = divmod(c, 4)
        out[b, j::4, :] = results[c]["y"]
    return _host_fixup(out, inputs)


def kernel(**inputs):
    from concourse.bass_utils import run_bass_kernel_spmd

    if "nc" not in _PROG_CACHE:
        _PROG_CACHE["nc"] = build_program()
    nc = _PROG_CACHE["nc"]
    in_maps = shard_inputs(inputs)
    res = run_bass_kernel_spmd(nc, in_maps, core_ids=list(range(8)))
    return unshard_outputs(res.results, inputs)


# revision 4
# speedup vs baseline: 1.0471x; 1.0022x over previous
"""GQA sparse-attention kernel for 8 Trainium2 NeuronCores.

Sharding: data-parallel over batch (2) x sequence-parallel over query rows
(rows j::4 interleaved, so causal work is balanced and the program is
SPMD-identical across cores). No collectives. Each core: q-proj for its 512
rows, full k/v proj for its batch, 16-head attention, layernorm + out-proj.

Key performance structure (vs the naive phase pipeline):
  - exact causal trimming of sim/PV/exp/mask work: per s-block k (128 wide),
    only query columns n >= 32k are computed (local row n <-> global 4n+j).
    Blocks are packed into PSUM tiles at bank-aligned offsets so exp/mask run
    on a few contiguous spans. sim+PV columns drop from 6144 to 4352 per head.
  - all-masked-row fallback moved to host (tiny numpy fixup on exact rows);
    the device lets those rows go NaN and the host overwrites them.
  - all input DMAs issued upfront as few big transfers (SP queue for the
    p1/p2 critical path, ACT queue for masks/small consts/p4 weights); adj
    arrives as prebuilt f16 mask tiles in the device block layout.
  - phase 3 software pipeline tuned to keep the PE instruction queue from
    ever stalling: the TRN2 tensor engine's clock drops to a mid p-state on
    any queue stall and needs ~3us of continuous execution to re-reach
    2.4GHz, so sim matmuls run 2-3 PSUM tiles ahead of the dependent PV
    matmuls (psim bufs=3) and each head's denominator tail (PE broadcast +
    DVE reciprocal + multiply) is issued one head late so its semaphore wait
    is pre-satisfied.
"""

import os
import sys

import numpy as np

for _p in ("/opt/trn_rl_repo", "/root/.axon_site/_ro/trn_rl_repo"):
    if os.path.isdir(_p) and _p not in sys.path:
        sys.path.insert(0, _p)

B, N, E = 2, 2048, 1024
HQ, HK, D = 16, 4, 64
G = HQ // HK          # 4 query heads per kv head
KVE = HK * D          # 256
NL = N // 4           # 512 local query rows per core
SB = N // 128         # 16 s-blocks
EC = E // 128         # 8 embedding chunks
LN_EPS = 1e-5

# s-block layout: block k -> (tile index, col offset, n0, width)
# tile A..E are [128,1024] (2 PSUM banks), F is [128,512] (1 bank)
BLK = [
    (0, 0, 0, 512), (0, 512, 32, 480),
    (1, 0, 64, 448), (1, 512, 96, 416),
    (2, 0, 128, 384), (2, 512, 160, 352),
    (3, 0, 192, 320), (3, 512, 224, 288),
    (4, 0, 256, 256), (4, 256, 288, 224), (4, 512, 320, 192), (4, 704, 352, 160),
    (5, 0, 384, 128), (5, 128, 416, 96), (5, 224, 448, 64), (5, 288, 480, 32),
]
# contiguous exp/mask spans per tile: (col_lo, col_hi)
SPANS = [
    [(0, 992)],
    [(0, 448), (512, 928)],
    [(0, 384), (512, 864)],
    [(0, 320), (512, 800)],
    [(0, 480), (512, 864)],
    [(0, 320)],
]
TILE_W = [1024, 1024, 1024, 1024, 1024, 512]
TILE_BLOCKS = [[0, 1], [2, 3], [4, 5], [6, 7], [8, 9, 10, 11], [12, 13, 14, 15]]

_PROG_CACHE = {}


def build_program():
    import concourse.mybir as mybir
    import concourse.tile as tile
    from concourse import bacc

    dt = mybir.dt
    f32, f32r, f16, i32 = dt.float32, dt.float32r, dt.float16, dt.int32
    AF = mybir.ActivationFunctionType
    OP = mybir.AluOpType
    AX = mybir.AxisListType

    nc = bacc.Bacc("TRN2", target_bir_lowering=False, debug=False)

    def din(name, shape, dtp=f32):
        return nc.dram_tensor(name, shape, dtp, kind="ExternalInput").ap()

    xqT = din("xqT", [E, NL], f16)
    xkT = din("xkT", [E, N], f16)
    xvT = din("xvT", [E, N], f16)
    WqT = din("WqT", [E, E], f16)          # pre-scaled by 1/8 on host
    WkT = din("WkT", [E, KVE], f16)
    WvT = din("WvT", [E, KVE], f16)
    WoT = din("WoT", [E, E], f16)
    bq2d = din("bq2d", [EC, 128])          # bq/8
    bk2d = din("bk2d", [2, 128])
    bv2d = din("bv2d", [2, 128])
    bo1 = din("bo1", [1, E], f16)
    lng = din("lng", [EC, 128])
    lnb = din("lnb", [EC, 128])
    ones1 = din("ones1", [1, 128], f32r)
    ones1h = din("ones1h", [1, 128], f16)
    masks_in = [din(f"mask{t}", [128, TILE_W[t]], f16) for t in range(6)]
    y = nc.dram_tensor("y", [NL, E], f32, kind="ExternalOutput").ap()

    with tile.TileContext(nc) as tc, nc.allow_low_precision(
            "f16/f32r operands for PE fast-path matmuls are intentional"):
        with (
            tc.tile_pool(name="const", bufs=1) as pc,
            tc.tile_pool(name="persist", bufs=1) as pp,
            tc.tile_pool(name="bigx", bufs=1) as pbx,
        ):
            # ---- upfront DMA prefetch ----
            # SP queue: p1 then p2 critical path (few big DMAs)
            wq_all = pc.tile([128, EC * E], f16, tag="wq_all")
            wq_sb = [wq_all[:, e * E:(e + 1) * E] for e in range(EC)]
            nc.sync.dma_start(
                wq_all[:].rearrange("p (e c) -> p e c", e=EC),
                WqT.rearrange("(e p) c -> p e c", e=EC))
            xq_all = pc.tile([128, EC * NL], f16, tag="xq_all")
            xq_sb = [xq_all[:, e * NL:(e + 1) * NL] for e in range(EC)]
            nc.sync.dma_start(
                xq_all[:].rearrange("p (e c) -> p e c", e=EC),
                xqT.rearrange("(e p) c -> p e c", e=EC))
            xk_all = pbx.tile([128, EC * N], f16, tag="xk_all")
            xv_all = pbx.tile([128, EC * N], f16, tag="xv_all")
            xk_sb = [xk_all[:, e * N:(e + 1) * N] for e in range(EC)]
            xv_sb = [xv_all[:, e * N:(e + 1) * N] for e in range(EC)]
            for st in range(4):
                ssl = slice(st * 512, (st + 1) * 512)
                nc.sync.dma_start(
                    xk_all[:].rearrange("p (e c) -> p e c", e=EC)[:, :, ssl],
                    xkT.rearrange("(e p) c -> p e c", e=EC)[:, :, ssl])
                nc.sync.dma_start(
                    xv_all[:].rearrange("p (e c) -> p e c", e=EC)[:, :, ssl],
                    xvT.rearrange("(e p) c -> p e c", e=EC)[:, :, ssl])
            # ACT queue: k/v weights, small consts, masks, p4 weights
            wk_all = pc.tile([128, EC * KVE], f16, tag="wk_all")
            wv_all = pc.tile([128, EC * KVE], f16, tag="wv_all")
            wk_sb = [wk_all[:, e * KVE:(e + 1) * KVE] for e in range(EC)]
            wv_sb = [wv_all[:, e * KVE:(e + 1) * KVE] for e in range(EC)]
            nc.scalar.dma_start(
                wk_all[:].rearrange("p (e c) -> p e c", e=EC),
                WkT.rearrange("(e p) c -> p e c", e=EC))
            nc.scalar.dma_start(
                wv_all[:].rearrange("p (e c) -> p e c", e=EC),
                WvT.rearrange("(e p) c -> p e c", e=EC))

            mask_sb = [pp.tile([128, TILE_W[t]], f16, tag=f"mk{t}", name=f"mk{t}")
                       for t in range(6)]
            for t in range(6):
                nc.scalar.dma_start(mask_sb[t][:], masks_in[t])
            bq_sb = [pc.tile([128, 1], f32, tag=f"bq{m}", name=f"bq{m}") for m in range(EC)]
            for e in range(EC):
                nc.scalar.dma_start(bq_sb[e][:], bq2d[e:e + 1, :])
            bk_sb = [pc.tile([128, 1], f32, tag=f"bk{m}", name=f"bk{m}") for m in range(2)]
            bv_sb = [pc.tile([128, 1], f32, tag=f"bv{m}", name=f"bv{m}") for m in range(2)]
            for m in range(2):
                nc.scalar.dma_start(bk_sb[m][:], bk2d[m:m + 1, :])
                nc.scalar.dma_start(bv_sb[m][:], bv2d[m:m + 1, :])
            ones_k1 = pc.tile([1, 128], f32r, tag="ones_k1")
            nc.scalar.dma_start(ones_k1[:], ones1)
            ones_m1 = pc.tile([128, 1], f16, tag="ones_m1")
            nc.scalar.dma_start(ones_m1[:], ones1h)
            ones_k1h = pc.tile([1, 128], f16, tag="ones_k1h")
            nc.scalar.dma_start(ones_k1h[:], ones1h)
            lng_sb = [pp.tile([128, 1], f32, tag=f"lng{e}", name=f"lng{e}") for e in range(EC)]
            lnb_sb = [pp.tile([128, 1], f32, tag=f"lnb{e}", name=f"lnb{e}") for e in range(EC)]
            for e in range(EC):
                nc.scalar.dma_start(lng_sb[e][:], lng[e:e + 1, :])
                nc.scalar.dma_start(lnb_sb[e][:], lnb[e:e + 1, :])
            bo_sb = pp.tile([1, E], f16, tag="bo", name="bo")
            nc.scalar.dma_start(bo_sb[:], bo1)
            wo_all = pp.tile([128, EC * E], f16, tag="wo_all", name="wo_all")
            wo_sb = [wo_all[:, e * E:(e + 1) * E] for e in range(EC)]
            nc.scalar.dma_start(
                wo_all[:].rearrange("p (e c) -> p e c", e=EC),
                WoT.rearrange("(e p) c -> p e c", e=EC))

            ident = pc.tile([128, 128], f16, tag="ident")
            from concourse.masks import make_identity
            make_identity(nc, ident[:])
            eps_c = pc.tile([1, 1], f32, tag="eps_c")
            nc.gpsimd.memset(eps_c[:], LN_EPS)
            ones64r = pc.tile([65, 64], f32r, tag="ones64r")
            nc.scalar.dma_start(ones64r[64:65, :], ones1[:, 0:64])

            # persistent activation tiles
            kT_sb = [pp.tile([128, N], f16, tag=f"kt{m}", name=f"kt{m}") for m in range(2)]
            v_ext = [pp.tile([128, 4 * 128], f16, tag=f"vx{k}", name=f"vx{k}") for k in range(SB)]
            qp_sb = [pp.tile([128, NL], f16, tag=f"qp{m}", name=f"qp{m}") for m in range(EC)]
            attT = [pp.tile([128, NL], f16, tag=f"at{e}", name=f"at{e}") for e in range(EC)]
            _EVEN = [0, 1, 2, 3, 8, 9, 10, 11]    # heads whose kv head is even
            _ODD = [4, 5, 6, 7, 12, 13, 14, 15]

            def _qslot(g):
                if (g // G) % 2 == 0:
                    return _EVEN.index(g), 0
                return _ODD.index(g), 1

            # ---------------- phase 1: q projection ----------------
            with tc.tile_pool(name="psq", bufs=2, space="PSUM") as psq:
                for mt2 in range(EC // 2):
                    psA = psq.tile([128, NL], f32, tag="psqA", name="psqA")
                    psB = psq.tile([128, NL], f32, tag="psqB", name="psqB")
                    for e in range(EC):
                        for mt, ps in ((2 * mt2, psA), (2 * mt2 + 1, psB)):
                            nc.tensor.matmul(
                                ps[:], wq_sb[e][:, mt * 128:(mt + 1) * 128],
                                xq_sb[e][:], start=(e == 0), stop=(e == EC - 1))
                    for mt, ps in ((2 * mt2, psA), (2 * mt2 + 1, psB)):
                        for t in range(2):
                            g = 2 * mt + t
                            ti, slot = _qslot(g)
                            nc.scalar.activation(
                                qp_sb[ti][slot * 64:(slot + 1) * 64, :],
                                ps[t * 64:(t + 1) * 64, :], AF.Identity,
                                bias=bq_sb[mt][t * 64:(t + 1) * 64, :], scale=1.0)

            # ---------------- phase 2: k/v projections ----------------
            with (
                tc.tile_pool(name="vt", bufs=2) as pvt,
                tc.tile_pool(name="pskv", bufs=2, space="PSUM") as pskv,
                tc.tile_pool(name="pst", bufs=2, space="PSUM") as pst,
            ):
                for st in range(4):  # s-tiles of 512
                    sl = slice(st * 512, (st + 1) * 512)
                    for mt in range(2):
                        psk = pskv.tile([128, 512], f32, tag="psk")
                        psv = pskv.tile([128, 512], f32, tag="psv")
                        for e in range(EC):
                            nc.tensor.matmul(
                                psk[:], wk_sb[e][:, mt * 128:(mt + 1) * 128],
                                xk_sb[e][:, sl], start=(e == 0), stop=(e == EC - 1))
                            nc.tensor.matmul(
                                psv[:], wv_sb[e][:, mt * 128:(mt + 1) * 128],
                                xv_sb[e][:, sl], start=(e == 0), stop=(e == EC - 1))
                        nc.scalar.activation(kT_sb[mt][:, sl], psk[:], AF.Identity,
                                             bias=bk_sb[mt][:], scale=1.0)
                        vt = pvt.tile([128, 512], f16, tag="vt")
                        nc.scalar.activation(vt[:], psv[:], AF.Identity,
                                             bias=bv_sb[mt][:], scale=1.0)
                        for ss in range(4):
                            k = st * 4 + ss
                            pt = pst.tile([128, 128], f16, tag="pt")
                            nc.tensor.transpose(pt[:], vt[:, ss * 128:(ss + 1) * 128],
                                                ident[:])
                            src = pt[:].rearrange("p (h x) -> p h x", h=2)
                            dst = v_ext[k][:].rearrange("p (h x) -> p h x", h=4)
                            nc.vector.tensor_copy(dst[:, 2 * mt:2 * mt + 2, 0:64], src)
                for k in range(SB):
                    pad = v_ext[k][:].rearrange("p (h x) -> p h x", h=4)[:, :, 64:128]
                    nc.gpsimd.memset(pad, 0.0)
                    one_col = v_ext[k][:].rearrange("p (h x) -> p h x", h=4)[:, :, 64:65]
                    nc.gpsimd.memset(one_col, 1.0)

            # ---------------- phase 3: attention ----------------
            with (
                tc.tile_pool(name="exs", bufs=5) as pex,
                tc.tile_pool(name="recs", bufs=2) as prec,
                tc.tile_pool(name="psim", bufs=3, space="PSUM") as psim,
                tc.tile_pool(name="pspv", bufs=1, space="PSUM") as pspv,
                tc.tile_pool(name="psbc", bufs=1, space="PSUM") as psbc,
            ):
                def head_work(g):
                    h = g // G
                    ti, slot = _qslot(g)
                    qg = qp_sb[ti][slot * 64:(slot + 1) * 64, :]
                    kh = kT_sb[h // 2][(h % 2) * 64:(h % 2) * 64 + 64, :]
                    pv = pspv.tile([128, 512], f32, tag="pv", name="pv")
                    exs = [None] * 6

                    def do_sim(t):
                        st_ = psim.tile([128, TILE_W[t]], f32, tag="sim", name="sim")
                        for k in TILE_BLOCKS[t]:
                            _, off, n0, w = BLK[k]
                            nc.tensor.matmul(
                                st_[:, off:off + w],
                                kh[:, k * 128:(k + 1) * 128], qg[:, n0:512],
                                start=True, stop=True)
                        ex = pex.tile([128, TILE_W[t]], f16, tag="ex", name="ex")
                        exs[t] = ex
                        for lo, hi in SPANS[t]:
                            nc.scalar.activation(ex[:, lo:hi], st_[:, lo:hi], AF.Exp)
                            nc.vector.tensor_tensor(
                                ex[:, lo:hi], ex[:, lo:hi], mask_sb[t][:, lo:hi],
                                op=OP.mult)

                    def do_pv(t):
                        ex = exs[t]
                        for k in TILE_BLOCKS[t]:
                            _, off, n0, w = BLK[k]
                            nc.tensor.matmul(
                                pv[:, n0:512], v_ext[k][:, 128 * h:128 * h + 128],
                                ex[:, off:off + w],
                                start=(k == 0), stop=(k == SB - 1),
                                skip_group_check=True)

                    def den_copy():
                        # den row (partition 64) to SBUF right after PV ends
                        den = prec.tile([65, 512], f32r, tag="den", name="den")
                        nc.vector.tensor_copy(den[64:65, :], pv[64:65, :])
                        return den

                    def tail(den):
                        # issued one head later so the PE's bc matmul never waits
                        bc = psbc.tile([64, 512], f32, tag="bc", name="bc")
                        nc.tensor.matmul(bc[:], ones64r[64:65, :], den[64:65, :],
                                         start=True, stop=True)
                        rec = prec.tile([64, 512], f32, tag="rec", name="rec")
                        nc.vector.reciprocal_approx_fast(rec[:], bc[:])
                        p0 = (g % 2) * 64
                        att = attT[g // 2][p0:p0 + 64, :]
                        nc.vector.tensor_tensor(att, pv[0:64, :], rec[:], op=OP.mult)

                    return do_sim, do_pv, den_copy, tail

                prev_tail = None
                for g in range(HQ):
                    do_sim, do_pv, den_copy, tail = head_work(g)
                    # software pipeline: sim runs 2-3 tiles ahead of pv;
                    # previous head's tail issues after this head's first sims.
                    do_sim(0)
                    do_sim(1)
                    if prev_tail is not None:
                        prev_tail()
                    do_sim(2)
                    do_sim(3)
                    do_pv(0)
                    do_pv(1)
                    do_sim(4)
                    do_pv(2)
                    do_pv(3)
                    do_sim(5)
                    do_pv(4)
                    do_pv(5)
                    den = den_copy()
                    prev_tail = (lambda t=tail, d=den: t(d))
                prev_tail()

            # ---------------- phase 4: layernorm + out projection ----------------
            with (
                tc.tile_pool(name="lnt", bufs=2) as plnt,
                tc.tile_pool(name="ysb", bufs=2) as pysb,
                tc.tile_pool(name="psst", bufs=1, space="PSUM") as psst,
                tc.tile_pool(name="pslb", bufs=1, space="PSUM") as pslb,
                tc.tile_pool(name="psy", bufs=2, space="PSUM") as psy,
            ):
                st_sum = psst.tile([1, NL], f32, tag="ssum")
                st_sq = psst.tile([1, NL], f32, tag="ssq")
                for e in range(EC):
                    nc.tensor.matmul(st_sum[:], ones_m1[:], attT[e][:],
                                     start=(e == 0), stop=(e == EC - 1))
                    sq = plnt.tile([128, NL], f16, tag="sq")
                    nc.scalar.activation(sq[:], attT[e][:], AF.Square)
                    nc.tensor.matmul(st_sq[:], ones_m1[:], sq[:],
                                     start=(e == 0), stop=(e == EC - 1))
                mu = plnt.tile([1, NL], f32r, tag="mu")
                nc.vector.tensor_scalar_mul(mu[:], st_sum[:], 1.0 / E)
                var = plnt.tile([1, NL], f32, tag="var")
                nc.vector.tensor_scalar_mul(var[:], st_sq[:], 1.0 / E)
                mu2 = plnt.tile([1, NL], f32, tag="mu2")
                nc.vector.tensor_tensor(mu2[:], mu[:], mu[:], op=OP.mult)
                nc.vector.tensor_tensor(var[:], var[:], mu2[:], op=OP.subtract)
                sd = plnt.tile([1, NL], f32, tag="sd")
                nc.scalar.activation(sd[:], var[:], AF.Sqrt, bias=eps_c[:])
                nc.vector.reciprocal_approx_fast(sd[:], sd[:])
                sdr = plnt.tile([1, NL], f32r, tag="sdr")
                nc.vector.tensor_copy(sdr[:], sd[:])
                mb = pslb.tile([128, NL], f32, tag="mb")
                nc.tensor.matmul(mb[:], ones_k1[:], mu[:], start=True, stop=True)
                ib = pslb.tile([128, NL], f32, tag="ib")
                nc.tensor.matmul(ib[:], ones_k1[:], sdr[:], start=True, stop=True)
                for e in range(EC):
                    tmp = plnt.tile([128, NL], f32, tag="xn")
                    nc.vector.tensor_tensor(tmp[:], attT[e][:], mb[:], op=OP.subtract)
                    nc.vector.tensor_tensor(tmp[:], tmp[:], ib[:], op=OP.mult)
                    nc.vector.tensor_scalar(attT[e][:], tmp[:], lng_sb[e][:],
                                            lnb_sb[e][:], op0=OP.mult, op1=OP.add)
                for nt in range(4):
                    pyA = psy.tile([128, 512], f32, tag="pyA", name="pyA")
                    pyB = psy.tile([128, 512], f32, tag="pyB", name="pyB")
                    pys = (pyA, pyB)
                    for e in range(EC):
                        for oc in range(2):
                            nc.tensor.matmul(
                                pys[oc][:], attT[e][:, nt * 128:(nt + 1) * 128],
                                wo_sb[e][:, oc * 512:(oc + 1) * 512],
                                start=(e == 0), stop=False)
                    for oc in range(2):
                        nc.tensor.matmul(pys[oc][:], ones_k1h[:],
                                         bo_sb[0:1, oc * 512:(oc + 1) * 512],
                                         start=False, stop=True)
                        ys = pysb.tile([128, 512], f32, tag="ys")
                        nc.vector.tensor_copy(ys[:], pys[oc][:])
                        nc.sync.dma_start(
                            y[nt * 128:(nt + 1) * 128, oc * 512:(oc + 1) * 512],
                            ys[:])
    nc.finalize()
    return nc


def _build_masks(adjc):
    """adjc: [NL, N] int (causal&adj premasked). Returns 6 f16 tiles in the
    block layout (tile cols <-> (block, n-range))."""
    at = adjc.T.astype(np.float16)  # [N, NL] = [s, n]
    tiles = [np.zeros((128, TILE_W[t]), np.float16) for t in range(6)]
    for k in range(SB):
        t, off, n0, w = BLK[k]
        tiles[t][:, off:off + w] = at[k * 128:(k + 1) * 128, n0:512]
    return tiles


def shard_inputs(inputs):
    q = np.asarray(inputs["query"], np.float32)
    k = np.asarray(inputs["key"], np.float32)
    v = np.asarray(inputs["value"], np.float32)
    adj = np.asarray(inputs["adj"], np.int32)
    WqT8 = (np.ascontiguousarray(np.asarray(inputs["Wq"], np.float32).T)
            / np.float32(8.0)).astype(np.float16)
    WkT = np.ascontiguousarray(np.asarray(inputs["Wk"], np.float32).T).astype(np.float16)
    WvT = np.ascontiguousarray(np.asarray(inputs["Wv"], np.float32).T).astype(np.float16)
    WoT = np.ascontiguousarray(np.asarray(inputs["Wo"], np.float32).T).astype(np.float16)
    bq8 = (np.asarray(inputs["bq"], np.float32) / np.float32(8.0)).reshape(EC, 128)
    bk2 = np.asarray(inputs["bk"], np.float32).reshape(2, 128)
    bv2 = np.asarray(inputs["bv"], np.float32).reshape(2, 128)
    bo1 = np.asarray(inputs["bo"], np.float32).reshape(1, E).astype(np.float16)
    lng = np.asarray(inputs["ln_g"], np.float32).reshape(EC, 128)
    lnb = np.asarray(inputs["ln_b"], np.float32).reshape(EC, 128)

    shared = dict(WqT=WqT8, WkT=WkT, WvT=WvT, WoT=WoT, bq2d=bq8, bk2d=bk2,
                  bv2d=bv2, bo1=bo1, lng=lng, lnb=lnb,
                  ones1=np.ones((1, 128), np.float32),
                  ones1h=np.ones((1, 128), np.float16))
    per_b = []
    s_idx = np.arange(N)
    for b in range(B):
        per_b.append((np.ascontiguousarray(k[b].T).astype(np.float16),
                      np.ascontiguousarray(v[b].T).astype(np.float16)))
    in_maps = []
    for c in range(8):
        b, j = divmod(c, 4)
        rows = np.arange(j, N, 4)
        causal = s_idx[None, :] <= rows[:, None]          # [NL, N]
        adjc = np.where(causal, adj[b][rows], 0)
        m = dict(shared)
        m["xqT"] = np.ascontiguousarray(q[b][rows].T).astype(np.float16)
        m["xkT"], m["xvT"] = per_b[b]
        for t, mk in enumerate(_build_masks(adjc)):
            m[f"mask{t}"] = mk
        in_maps.append(m)
    return in_maps


def _host_fixup(out, inputs):
    """Rows with no unmasked causal position get the reference's uniform-
    softmax-over-everything fallback, computed exactly on host."""
    adj = np.asarray(inputs["adj"])
    s_idx = np.arange(N)
    causal = s_idx[None, :] <= s_idx[:, None]
    for b in range(B):
        amr = np.where(((adj[b] != 0) & causal).sum(1) == 0)[0]
        if len(amr) == 0:
            continue
        v = np.asarray(inputs["value"][b], np.float64)
        Wv = np.asarray(inputs["Wv"], np.float64)
        bv = np.asarray(inputs["bv"], np.float64)
        vp = v @ Wv.T + bv                       # [N, KVE]
        mv = vp.mean(0)                          # [KVE]
        row = np.concatenate([mv[(k // G) * D:(k // G) * D + D] for k in range(HQ)])
        mu = row.mean()
        var = ((row - mu) ** 2).mean()
        rown = (row - mu) / np.sqrt(var + LN_EPS)
        rown = rown * np.asarray(inputs["ln_g"], np.float64) + np.asarray(
            inputs["ln_b"], np.float64)
        yrow = rown @ np.asarray(inputs["Wo"], np.float64).T + np.asarray(
            inputs["bo"], np.float64)
        out[b, amr, :] = yrow.astype(np.float32)
    return out


def unshard_outputs(results, inputs):
    out = np.empty((B, N, E), np.float32)
    for c in range(8):
        b, j = divmod(c, 4)
        out[b, j::4, :] = results[c]["y"]
    return _host_fixup(out, inputs)


def kernel(**inputs):
    from concourse.bass_utils import run_bass_kernel_spmd

    if "nc" not in _PROG_CACHE:
        _PROG_CACHE["nc"] = build_program()
    nc = _PROG_CACHE["nc"]
    in_maps = shard_inputs(inputs)
    res = run_bass_kernel_spmd(nc, in_maps, core_ids=list(range(8)))
    return unshard_outputs(res.results, inputs)


# revision 5
# speedup vs baseline: 1.0899x; 1.0409x over previous
"""GQA sparse-attention kernel for 8 Trainium2 NeuronCores — v2.

Sharding: data-parallel over batch (2) x sequence-parallel over query rows
(rows j::4 interleaved). No collectives.

v2 changes vs baseline:
  - exact causal trimming of sim/PV/exp/mask work: per s-block k (128 wide),
    only query columns n >= 32k are computed (local row n <-> global 4n+j).
    Blocks are packed into PSUM tiles at bank-aligned offsets so exp/mask run
    on a few contiguous spans. sim+PV columns drop from 6144 to 4352 per head.
  - all-masked-row fallback moved to host (tiny numpy fixup on exact rows);
    removes the em_* device pipeline.
  - all input DMAs issued upfront (SP queue for the p1/p2 critical path, ACT
    queue for masks/p4 weights); adj arrives as prebuilt f16 mask tiles.
  - per-head tail: PE broadcast of denominator + DVE reciprocal + multiply.
  - software-pipelined per-head issue order (sim runs 2-3 tiles ahead of PV).
"""

import os
import sys

import numpy as np

for _p in ("/opt/trn_rl_repo", "/root/.axon_site/_ro/trn_rl_repo"):
    if os.path.isdir(_p) and _p not in sys.path:
        sys.path.insert(0, _p)

B, N, E = 2, 2048, 1024
HQ, HK, D = 16, 4, 64
G = HQ // HK          # 4 query heads per kv head
KVE = HK * D          # 256
NL = N // 4           # 512 local query rows per core
SB = N // 128         # 16 s-blocks
EC = E // 128         # 8 embedding chunks
LN_EPS = 1e-5

# s-block layout: block k -> (tile index, col offset, n0, width)
# tile A..E are [128,1024] (2 PSUM banks), F is [128,512] (1 bank)
BLK = [
    (0, 0, 0, 512), (0, 512, 32, 480),
    (1, 0, 64, 448), (1, 512, 96, 416),
    (2, 0, 128, 384), (2, 512, 160, 352),
    (3, 0, 192, 320), (3, 512, 224, 288),
    (4, 0, 256, 256), (4, 256, 288, 224), (4, 512, 320, 192), (4, 704, 352, 160),
    (5, 0, 384, 128), (5, 128, 416, 96), (5, 224, 448, 64), (5, 288, 480, 32),
]
# contiguous exp/mask spans per tile: (col_lo, col_hi)
SPANS = [
    [(0, 992)],
    [(0, 448), (512, 928)],
    [(0, 384), (512, 864)],
    [(0, 320), (512, 800)],
    [(0, 480), (512, 864)],
    [(0, 320)],
]
TILE_W = [1024, 1024, 1024, 1024, 1024, 512]
TILE_BLOCKS = [[0, 1], [2, 3], [4, 5], [6, 7], [8, 9, 10, 11], [12, 13, 14, 15]]

_PROG_CACHE = {}


def build_program():
    import concourse.mybir as mybir
    import concourse.tile as tile
    from concourse import bacc

    dt = mybir.dt
    f32, f32r, f16, i32 = dt.float32, dt.float32r, dt.float16, dt.int32
    AF = mybir.ActivationFunctionType
    OP = mybir.AluOpType
    AX = mybir.AxisListType

    nc = bacc.Bacc("TRN2", target_bir_lowering=False, debug=False)

    def din(name, shape, dtp=f32):
        return nc.dram_tensor(name, shape, dtp, kind="ExternalInput").ap()

    xqT = din("xqT", [E, NL], f16)
    xkT = din("xkT", [E, N], f16)
    xvT = din("xvT", [E, N], f16)
    WqT = din("WqT", [E, E], f16)          # pre-scaled by 1/8 on host
    WkT = din("WkT", [E, KVE], f16)
    WvT = din("WvT", [E, KVE], f16)
    WoT = din("WoT", [E, E], f16)
    bq2d = din("bq2d", [EC, 128])          # bq/8
    bk2d = din("bk2d", [2, 128])
    bv2d = din("bv2d", [2, 128])
    bo1 = din("bo1", [1, E], f16)
    lng = din("lng", [EC, 128])
    lnb = din("lnb", [EC, 128])
    ones1 = din("ones1", [1, 128], f32r)
    ones1h = din("ones1h", [1, 128], f16)
    masks_in = [din(f"mask{t}", [128, TILE_W[t]], f16) for t in range(6)]
    y = nc.dram_tensor("y", [NL, E], f32, kind="ExternalOutput").ap()

    with tile.TileContext(nc) as tc, nc.allow_low_precision(
            "f16/f32r operands for PE fast-path matmuls are intentional"):
        with (
            tc.tile_pool(name="const", bufs=1) as pc,
            tc.tile_pool(name="persist", bufs=1) as pp,
            tc.tile_pool(name="bigx", bufs=1) as pbx,
        ):
            # ---- upfront DMA prefetch ----
            # SP queue: p1 then p2 critical path (few big DMAs)
            xk_sb = [pbx.tile([128, N], f16, tag=f"xk{e}", name=f"xk{e}") for e in range(EC)]
            xv_sb = [pbx.tile([128, N], f16, tag=f"xv{e}", name=f"xv{e}") for e in range(EC)]
            wq_sb = [pc.tile([128, E], f16, tag=f"wq{e}", name=f"wq{e}") for e in range(EC)]
            xq_sb = [pc.tile([128, NL], f16, tag=f"xq{e}", name=f"xq{e}") for e in range(EC)]
            for e in range(EC):
                nc.sync.dma_start(xk_sb[e][:], xkT[e * 128:(e + 1) * 128, :])
            for e in range(EC):
                nc.sync.dma_start(wq_sb[e][:], WqT[e * 128:(e + 1) * 128, :])
            for e in range(EC):
                nc.sync.dma_start(xq_sb[e][:], xqT[e * 128:(e + 1) * 128, :])
            for e in range(EC):
                nc.sync.dma_start(xv_sb[e][:], xvT[e * 128:(e + 1) * 128, :])
            # ACT queue: k/v weights, small consts, masks, p4 weights
            wk_all = pc.tile([128, EC * KVE], f16, tag="wk_all")
            wv_all = pc.tile([128, EC * KVE], f16, tag="wv_all")
            wk_sb = [wk_all[:, e * KVE:(e + 1) * KVE] for e in range(EC)]
            wv_sb = [wv_all[:, e * KVE:(e + 1) * KVE] for e in range(EC)]
            nc.scalar.dma_start(
                wk_all[:].rearrange("p (e c) -> p e c", e=EC),
                WkT.rearrange("(e p) c -> p e c", e=EC))
            nc.scalar.dma_start(
                wv_all[:].rearrange("p (e c) -> p e c", e=EC),
                WvT.rearrange("(e p) c -> p e c", e=EC))

            mask_sb = [pp.tile([128, TILE_W[t]], f16, tag=f"mk{t}", name=f"mk{t}")
                       for t in range(6)]
            for t in range(6):
                nc.scalar.dma_start(mask_sb[t][:], masks_in[t])
            bq_sb = [pc.tile([128, 1], f32, tag=f"bq{m}", name=f"bq{m}") for m in range(EC)]
            for e in range(EC):
                nc.scalar.dma_start(bq_sb[e][:], bq2d[e:e + 1, :])
            bk_sb = [pc.tile([128, 1], f32, tag=f"bk{m}", name=f"bk{m}") for m in range(2)]
            bv_sb = [pc.tile([128, 1], f32, tag=f"bv{m}", name=f"bv{m}") for m in range(2)]
            for m in range(2):
                nc.scalar.dma_start(bk_sb[m][:], bk2d[m:m + 1, :])
                nc.scalar.dma_start(bv_sb[m][:], bv2d[m:m + 1, :])
            ones_k1 = pc.tile([1, 128], f32r, tag="ones_k1")
            nc.scalar.dma_start(ones_k1[:], ones1)
            ones_m1 = pc.tile([128, 1], f16, tag="ones_m1")
            nc.scalar.dma_start(ones_m1[:], ones1h)
            ones_k1h = pc.tile([1, 128], f16, tag="ones_k1h")
            nc.scalar.dma_start(ones_k1h[:], ones1h)
            lng_sb = [pp.tile([128, 1], f32, tag=f"lng{e}", name=f"lng{e}") for e in range(EC)]
            lnb_sb = [pp.tile([128, 1], f32, tag=f"lnb{e}", name=f"lnb{e}") for e in range(EC)]
            for e in range(EC):
                nc.scalar.dma_start(lng_sb[e][:], lng[e:e + 1, :])
                nc.scalar.dma_start(lnb_sb[e][:], lnb[e:e + 1, :])
            bo_sb = pp.tile([1, E], f16, tag="bo", name="bo")
            nc.scalar.dma_start(bo_sb[:], bo1)
            wo_all = pp.tile([128, EC * E], f16, tag="wo_all", name="wo_all")
            wo_sb = [wo_all[:, e * E:(e + 1) * E] for e in range(EC)]
            nc.scalar.dma_start(
                wo_all[:].rearrange("p (e c) -> p e c", e=EC),
                WoT.rearrange("(e p) c -> p e c", e=EC))

            ident = pc.tile([128, 128], f16, tag="ident")
            from concourse.masks import make_identity
            make_identity(nc, ident[:])
            eps_c = pc.tile([1, 1], f32, tag="eps_c")
            nc.gpsimd.memset(eps_c[:], LN_EPS)
            ones64r = pc.tile([65, 64], f32r, tag="ones64r")
            nc.scalar.dma_start(ones64r[64:65, :], ones1[:, 0:64])

            # persistent activation tiles
            kT_sb = [pp.tile([128, N], f16, tag=f"kt{m}", name=f"kt{m}") for m in range(2)]
            v_ext = [pp.tile([128, 4 * 128], f16, tag=f"vx{k}", name=f"vx{k}") for k in range(SB)]
            qp_sb = [pp.tile([128, NL], f16, tag=f"qp{m}", name=f"qp{m}") for m in range(EC)]
            attT = [pp.tile([128, NL], f16, tag=f"at{e}", name=f"at{e}") for e in range(EC)]
            _EVEN = [0, 1, 2, 3, 8, 9, 10, 11]    # heads whose kv head is even
            _ODD = [4, 5, 6, 7, 12, 13, 14, 15]

            def _qslot(g):
                if (g // G) % 2 == 0:
                    return _EVEN.index(g), 0
                return _ODD.index(g), 1

            # ---------------- phase 0: k projection (xk+wk arrive first) ---
            with tc.tile_pool(name="psk2", bufs=2, space="PSUM") as psk2:
                for st in range(4):
                    sl = slice(st * 512, (st + 1) * 512)
                    for mt in range(2):
                        psk = psk2.tile([128, 512], f32, tag="psk")
                        for e in range(EC):
                            nc.tensor.matmul(
                                psk[:], wk_sb[e][:, mt * 128:(mt + 1) * 128],
                                xk_sb[e][:, sl], start=(e == 0), stop=(e == EC - 1))
                        nc.scalar.activation(kT_sb[mt][:, sl], psk[:], AF.Identity,
                                             bias=bk_sb[mt][:], scale=1.0)

            # ---------------- phase 1: q projection ----------------
            with tc.tile_pool(name="psq", bufs=2, space="PSUM") as psq:
                for mt2 in range(EC // 2):
                    psA = psq.tile([128, NL], f32, tag="psqA", name="psqA")
                    psB = psq.tile([128, NL], f32, tag="psqB", name="psqB")
                    for e in range(EC):
                        for mt, ps in ((2 * mt2, psA), (2 * mt2 + 1, psB)):
                            nc.tensor.matmul(
                                ps[:], wq_sb[e][:, mt * 128:(mt + 1) * 128],
                                xq_sb[e][:], start=(e == 0), stop=(e == EC - 1))
                    for mt, ps in ((2 * mt2, psA), (2 * mt2 + 1, psB)):
                        for t in range(2):
                            g = 2 * mt + t
                            ti, slot = _qslot(g)
                            nc.scalar.activation(
                                qp_sb[ti][slot * 64:(slot + 1) * 64, :],
                                ps[t * 64:(t + 1) * 64, :], AF.Identity,
                                bias=bq_sb[mt][t * 64:(t + 1) * 64, :], scale=1.0)

            # ---------------- phase 2: k/v projections ----------------
            with (
                tc.tile_pool(name="vt", bufs=2) as pvt,
                tc.tile_pool(name="pskv", bufs=2, space="PSUM") as pskv,
                tc.tile_pool(name="pst", bufs=2, space="PSUM") as pst,
            ):
                for st in range(4):  # s-tiles of 512
                    sl = slice(st * 512, (st + 1) * 512)
                    for mt in range(2):
                        psv = pskv.tile([128, 512], f32, tag="psv")
                        for e in range(EC):
                            nc.tensor.matmul(
                                psv[:], wv_sb[e][:, mt * 128:(mt + 1) * 128],
                                xv_sb[e][:, sl], start=(e == 0), stop=(e == EC - 1))
                        vt = pvt.tile([128, 512], f16, tag="vt")
                        nc.scalar.activation(vt[:], psv[:], AF.Identity,
                                             bias=bv_sb[mt][:], scale=1.0)
                        for ss in range(4):
                            k = st * 4 + ss
                            pt = pst.tile([128, 128], f16, tag="pt")
                            nc.tensor.transpose(pt[:], vt[:, ss * 128:(ss + 1) * 128],
                                                ident[:])
                            src = pt[:].rearrange("p (h x) -> p h x", h=2)
                            dst = v_ext[k][:].rearrange("p (h x) -> p h x", h=4)
                            nc.vector.tensor_copy(dst[:, 2 * mt:2 * mt + 2, 0:64], src)
                for k in range(SB):
                    pad = v_ext[k][:].rearrange("p (h x) -> p h x", h=4)[:, :, 64:128]
                    nc.gpsimd.memset(pad, 0.0)
                    one_col = v_ext[k][:].rearrange("p (h x) -> p h x", h=4)[:, :, 64:65]
                    nc.gpsimd.memset(one_col, 1.0)

            # ---------------- phase 3: attention ----------------
            with (
                tc.tile_pool(name="exs", bufs=5) as pex,
                tc.tile_pool(name="recs", bufs=2) as prec,
                tc.tile_pool(name="psim", bufs=3, space="PSUM") as psim,
                tc.tile_pool(name="pspv", bufs=1, space="PSUM") as pspv,
                tc.tile_pool(name="psbc", bufs=1, space="PSUM") as psbc,
            ):
                def head_work(g):
                    h = g // G
                    ti, slot = _qslot(g)
                    qg = qp_sb[ti][slot * 64:(slot + 1) * 64, :]
                    kh = kT_sb[h // 2][(h % 2) * 64:(h % 2) * 64 + 64, :]
                    pv = pspv.tile([128, 512], f32, tag="pv", name="pv")
                    exs = [None] * 6

                    def do_sim(t):
                        st_ = psim.tile([128, TILE_W[t]], f32, tag="sim", name="sim")
                        for k in TILE_BLOCKS[t]:
                            _, off, n0, w = BLK[k]
                            nc.tensor.matmul(
                                st_[:, off:off + w],
                                kh[:, k * 128:(k + 1) * 128], qg[:, n0:512],
                                start=True, stop=True)
                        ex = pex.tile([128, TILE_W[t]], f16, tag="ex", name="ex")
                        exs[t] = ex
                        for lo, hi in SPANS[t]:
                            nc.scalar.activation(ex[:, lo:hi], st_[:, lo:hi], AF.Exp)
                            nc.vector.tensor_tensor(
                                ex[:, lo:hi], ex[:, lo:hi], mask_sb[t][:, lo:hi],
                                op=OP.mult)

                    def do_pv(t):
                        ex = exs[t]
                        for k in TILE_BLOCKS[t]:
                            _, off, n0, w = BLK[k]
                            nc.tensor.matmul(
                                pv[:, n0:512], v_ext[k][:, 128 * h:128 * h + 128],
                                ex[:, off:off + w],
                                start=(k == 0), stop=(k == SB - 1),
                                skip_group_check=True)

                    def den_copy():
                        # den row (partition 64) to SBUF right after PV ends
                        den = prec.tile([65, 512], f32r, tag="den", name="den")
                        nc.vector.tensor_copy(den[64:65, :], pv[64:65, :])
                        return den

                    def tail(den):
                        # issued one head later so the PE's bc matmul never waits
                        bc = psbc.tile([64, 512], f32, tag="bc", name="bc")
                        nc.tensor.matmul(bc[:], ones64r[64:65, :], den[64:65, :],
                                         start=True, stop=True)
                        rec = prec.tile([64, 512], f32, tag="rec", name="rec")
                        nc.vector.reciprocal_approx_fast(rec[:], bc[:])
                        p0 = (g % 2) * 64
                        att = attT[g // 2][p0:p0 + 64, :]
                        nc.vector.tensor_tensor(att, pv[0:64, :], rec[:], op=OP.mult)

                    return do_sim, do_pv, den_copy, tail

                prev_tail = None
                for g in range(HQ):
                    do_sim, do_pv, den_copy, tail = head_work(g)
                    # software pipeline: sim runs 2-3 tiles ahead of pv;
                    # previous head's tail issues after this head's first sims.
                    do_sim(0)
                    do_sim(1)
                    if prev_tail is not None:
                        prev_tail()
                    do_sim(2)
                    do_sim(3)
                    do_pv(0)
                    do_pv(1)
                    do_sim(4)
                    do_pv(2)
                    do_pv(3)
                    do_sim(5)
                    do_pv(4)
                    do_pv(5)
                    den = den_copy()
                    prev_tail = (lambda t=tail, d=den: t(d))
                prev_tail()

            # ---------------- phase 4: layernorm + out projection ----------------
            with (
                tc.tile_pool(name="lnt", bufs=2) as plnt,
                tc.tile_pool(name="ysb", bufs=2) as pysb,
                tc.tile_pool(name="psst", bufs=1, space="PSUM") as psst,
                tc.tile_pool(name="pslb", bufs=1, space="PSUM") as pslb,
                tc.tile_pool(name="psy", bufs=2, space="PSUM") as psy,
            ):
                st_sum = psst.tile([1, NL], f32, tag="ssum")
                st_sq = psst.tile([1, NL], f32, tag="ssq")
                for e in range(EC):
                    nc.tensor.matmul(st_sum[:], ones_m1[:], attT[e][:],
                                     start=(e == 0), stop=(e == EC - 1))
                    sq = plnt.tile([128, NL], f16, tag="sq")
                    nc.scalar.activation(sq[:], attT[e][:], AF.Square)
                    nc.tensor.matmul(st_sq[:], ones_m1[:], sq[:],
                                     start=(e == 0), stop=(e == EC - 1))
                mu = plnt.tile([1, NL], f32r, tag="mu")
                nc.vector.tensor_scalar_mul(mu[:], st_sum[:], 1.0 / E)
                var = plnt.tile([1, NL], f32, tag="var")
                nc.vector.tensor_scalar_mul(var[:], st_sq[:], 1.0 / E)
                mu2 = plnt.tile([1, NL], f32, tag="mu2")
                nc.vector.tensor_tensor(mu2[:], mu[:], mu[:], op=OP.mult)
                nc.vector.tensor_tensor(var[:], var[:], mu2[:], op=OP.subtract)
                sd = plnt.tile([1, NL], f32, tag="sd")
                nc.scalar.activation(sd[:], var[:], AF.Sqrt, bias=eps_c[:])
                nc.vector.reciprocal_approx_fast(sd[:], sd[:])
                sdr = plnt.tile([1, NL], f32r, tag="sdr")
                nc.vector.tensor_copy(sdr[:], sd[:])
                mb = pslb.tile([128, NL], f32, tag="mb")
                nc.tensor.matmul(mb[:], ones_k1[:], mu[:], start=True, stop=True)
                ib = pslb.tile([128, NL], f32, tag="ib")
                nc.tensor.matmul(ib[:], ones_k1[:], sdr[:], start=True, stop=True)
                for e in range(EC):
                    tmp = plnt.tile([128, NL], f32, tag="xn")
                    nc.vector.tensor_tensor(tmp[:], attT[e][:], mb[:], op=OP.subtract)
                    nc.vector.tensor_tensor(tmp[:], tmp[:], ib[:], op=OP.mult)
                    nc.vector.tensor_scalar(attT[e][:], tmp[:], lng_sb[e][:],
                                            lnb_sb[e][:], op0=OP.mult, op1=OP.add)
                for nt in range(4):
                    pyA = psy.tile([128, 512], f32, tag="pyA", name="pyA")
                    pyB = psy.tile([128, 512], f32, tag="pyB", name="pyB")
                    pys = (pyA, pyB)
                    for e in range(EC):
                        for oc in range(2):
                            nc.tensor.matmul(
                                pys[oc][:], attT[e][:, nt * 128:(nt + 1) * 128],
                                wo_sb[e][:, oc * 512:(oc + 1) * 512],
                                start=(e == 0), stop=False)
                    for oc in range(2):
                        nc.tensor.matmul(pys[oc][:], ones_k1h[:],
                                         bo_sb[0:1, oc * 512:(oc + 1) * 512],
                                         start=False, stop=True)
                        ys = pysb.tile([128, 512], f32, tag="ys")
                        nc.vector.tensor_copy(ys[:], pys[oc][:])
                        nc.sync.dma_start(
                            y[nt * 128:(nt + 1) * 128, oc * 512:(oc + 1) * 512],
                            ys[:])
    nc.finalize()
    return nc


def _build_masks(adjc):
    """adjc: [NL, N] int (causal&adj premasked). Returns 6 f16 tiles in the
    block layout (tile cols <-> (block, n-range))."""
    at = adjc.T.astype(np.float16)  # [N, NL] = [s, n]
    tiles = [np.zeros((128, TILE_W[t]), np.float16) for t in range(6)]
    for k in range(SB):
        t, off, n0, w = BLK[k]
        tiles[t][:, off:off + w] = at[k * 128:(k + 1) * 128, n0:512]
    return tiles


def shard_inputs(inputs):
    q = np.asarray(inputs["query"], np.float32)
    k = np.asarray(inputs["key"], np.float32)
    v = np.asarray(inputs["value"], np.float32)
    adj = np.asarray(inputs["adj"], np.int32)
    WqT8 = (np.ascontiguousarray(np.asarray(inputs["Wq"], np.float32).T)
            / np.float32(8.0)).astype(np.float16)
    WkT = np.ascontiguousarray(np.asarray(inputs["Wk"], np.float32).T).astype(np.float16)
    WvT = np.ascontiguousarray(np.asarray(inputs["Wv"], np.float32).T).astype(np.float16)
    WoT = np.ascontiguousarray(np.asarray(inputs["Wo"], np.float32).T).astype(np.float16)
    bq8 = (np.asarray(inputs["bq"], np.float32) / np.float32(8.0)).reshape(EC, 128)
    bk2 = np.asarray(inputs["bk"], np.float32).reshape(2, 128)
    bv2 = np.asarray(inputs["bv"], np.float32).reshape(2, 128)
    bo1 = np.asarray(inputs["bo"], np.float32).reshape(1, E).astype(np.float16)
    lng = np.asarray(inputs["ln_g"], np.float32).reshape(EC, 128)
    lnb = np.asarray(inputs["ln_b"], np.float32).reshape(EC, 128)

    shared = dict(WqT=WqT8, WkT=WkT, WvT=WvT, WoT=WoT, bq2d=bq8, bk2d=bk2,
                  bv2d=bv2, bo1=bo1, lng=lng, lnb=lnb,
                  ones1=np.ones((1, 128), np.float32),
                  ones1h=np.ones((1, 128), np.float16))
    per_b = []
    s_idx = np.arange(N)
    for b in range(B):
        per_b.append((np.ascontiguousarray(k[b].T).astype(np.float16),
                      np.ascontiguousarray(v[b].T).astype(np.float16)))
    in_maps = []
    for c in range(8):
        b, j = divmod(c, 4)
        rows = np.arange(j, N, 4)
        causal = s_idx[None, :] <= rows[:, None]          # [NL, N]
        adjc = np.where(causal, adj[b][rows], 0)
        m = dict(shared)
        m["xqT"] = np.ascontiguousarray(q[b][rows].T).astype(np.float16)
        m["xkT"], m["xvT"] = per_b[b]
        for t, mk in enumerate(_build_masks(adjc)):
            m[f"mask{t}"] = mk
        in_maps.append(m)
    return in_maps


def _host_fixup(out, inputs):
    """Rows with no unmasked causal position get the reference's uniform-
    softmax-over-everything fallback, computed exactly on host."""
    adj = np.asarray(inputs["adj"])
    s_idx = np.arange(N)
    causal = s_idx[None, :] <= s_idx[:, None]
    for b in range(B):
        amr = np.where(((adj[b] != 0) & causal).sum(1) == 0)[0]
        if len(amr) == 0:
            continue
        v = np.asarray(inputs["value"][b], np.float64)
        Wv = np.asarray(inputs["Wv"], np.float64)
        bv = np.asarray(inputs["bv"], np.float64)
        vp = v @ Wv.T + bv                       # [N, KVE]
        mv = vp.mean(0)                          # [KVE]
        row = np.concatenate([mv[(k // G) * D:(k // G) * D + D] for k in range(HQ)])
        mu = row.mean()
        var = ((row - mu) ** 2).mean()
        rown = (row - mu) / np.sqrt(var + LN_EPS)
        rown = rown * np.asarray(inputs["ln_g"], np.float64) + np.asarray(
            inputs["ln_b"], np.float64)
        yrow = rown @ np.asarray(inputs["Wo"], np.float64).T + np.asarray(
            inputs["bo"], np.float64)
        out[b, amr, :] = yrow.astype(np.float32)
    return out


def unshard_outputs(results, inputs):
    out = np.empty((B, N, E), np.float32)
    for c in range(8):
        b, j = divmod(c, 4)
        out[b, j::4, :] = results[c]["y"]
    return _host_fixup(out, inputs)


def kernel(**inputs):
    from concourse.bass_utils import run_bass_kernel_spmd

    if "nc" not in _PROG_CACHE:
        _PROG_CACHE["nc"] = build_program()
    nc = _PROG_CACHE["nc"]
    in_maps = shard_inputs(inputs)
    res = run_bass_kernel_spmd(nc, in_maps, core_ids=list(range(8)))
    return unshard_outputs(res.results, inputs)


# revision 6
# speedup vs baseline: 1.0902x; 1.0003x over previous
"""GQA sparse-attention kernel for 8 Trainium2 NeuronCores — v2.

Sharding: data-parallel over batch (2) x sequence-parallel over query rows
(rows j::4 interleaved). No collectives.

v2 changes vs baseline:
  - exact causal trimming of sim/PV/exp/mask work: per s-block k (128 wide),
    only query columns n >= 32k are computed (local row n <-> global 4n+j).
    Blocks are packed into PSUM tiles at bank-aligned offsets so exp/mask run
    on a few contiguous spans. sim+PV columns drop from 6144 to 4352 per head.
  - all-masked-row fallback moved to host (tiny numpy fixup on exact rows);
    removes the em_* device pipeline.
  - all input DMAs issued upfront (SP queue for the p1/p2 critical path, ACT
    queue for masks/p4 weights); adj arrives as prebuilt f16 mask tiles.
  - per-head tail: PE broadcast of denominator + DVE reciprocal + multiply.
  - software-pipelined per-head issue order (sim runs 2-3 tiles ahead of PV).
"""

import os
import sys

import numpy as np

for _p in ("/opt/trn_rl_repo", "/root/.axon_site/_ro/trn_rl_repo"):
    if os.path.isdir(_p) and _p not in sys.path:
        sys.path.insert(0, _p)

B, N, E = 2, 2048, 1024
HQ, HK, D = 16, 4, 64
G = HQ // HK          # 4 query heads per kv head
KVE = HK * D          # 256
NL = N // 4           # 512 local query rows per core
SB = N // 128         # 16 s-blocks
EC = E // 128         # 8 embedding chunks
LN_EPS = 1e-5

# s-block layout: block k -> (tile index, col offset, n0, width)
# tile A..E are [128,1024] (2 PSUM banks), F is [128,512] (1 bank)
BLK = [
    (0, 0, 0, 512), (0, 512, 32, 480),
    (1, 0, 64, 448), (1, 512, 96, 416),
    (2, 0, 128, 384), (2, 512, 160, 352),
    (3, 0, 192, 320), (3, 512, 224, 288),
    (4, 0, 256, 256), (4, 256, 288, 224), (4, 512, 320, 192), (4, 704, 352, 160),
    (5, 0, 384, 128), (5, 128, 416, 96), (5, 224, 448, 64), (5, 288, 480, 32),
]
# contiguous exp/mask spans per tile: (col_lo, col_hi)
SPANS = [
    [(0, 992)],
    [(0, 448), (512, 928)],
    [(0, 384), (512, 864)],
    [(0, 320), (512, 800)],
    [(0, 480), (512, 864)],
    [(0, 320)],
]
TILE_W = [1024, 1024, 1024, 1024, 1024, 512]
TILE_BLOCKS = [[0, 1], [2, 3], [4, 5], [6, 7], [8, 9, 10, 11], [12, 13, 14, 15]]

_PROG_CACHE = {}


def build_program():
    import concourse.mybir as mybir
    import concourse.tile as tile
    from concourse import bacc

    dt = mybir.dt
    f32, f32r, f16, i32 = dt.float32, dt.float32r, dt.float16, dt.int32
    AF = mybir.ActivationFunctionType
    OP = mybir.AluOpType
    AX = mybir.AxisListType

    nc = bacc.Bacc("TRN2", target_bir_lowering=False, debug=False)

    def din(name, shape, dtp=f32):
        return nc.dram_tensor(name, shape, dtp, kind="ExternalInput").ap()

    xqT = din("xqT", [E, NL], f16)
    xkT = din("xkT", [E, N], f16)
    xvT = din("xvT", [E, N], f16)
    WqT = din("WqT", [E, E], f16)          # pre-scaled by 1/8 on host
    WkT = din("WkT", [E, KVE], f16)
    WvT = din("WvT", [E, KVE], f16)
    WoT = din("WoT", [E, E], f16)
    bq2d = din("bq2d", [EC, 128])          # bq/8
    bk2d = din("bk2d", [2, 128])
    bv2d = din("bv2d", [2, 128])
    bo1 = din("bo1", [1, E], f16)
    lng = din("lng", [EC, 128])
    lnb = din("lnb", [EC, 128])
    ones1 = din("ones1", [1, 128], f32r)
    ones1h = din("ones1h", [1, 128], f16)
    masks_in = [din(f"mask{t}", [128, TILE_W[t]], f16) for t in range(6)]
    y = nc.dram_tensor("y", [NL, E], f32, kind="ExternalOutput").ap()

    with tile.TileContext(nc) as tc, nc.allow_low_precision(
            "f16/f32r operands for PE fast-path matmuls are intentional"):
        with (
            tc.tile_pool(name="const", bufs=1) as pc,
            tc.tile_pool(name="persist", bufs=1) as pp,
            tc.tile_pool(name="bigx", bufs=1) as pbx,
        ):
            # ---- upfront DMA prefetch ----
            # SP queue: p1 then p2 critical path (few big DMAs)
            xk_sb = [pbx.tile([128, N], f16, tag=f"xk{e}", name=f"xk{e}") for e in range(EC)]
            xv_sb = [pbx.tile([128, N], f16, tag=f"xv{e}", name=f"xv{e}") for e in range(EC)]
            wq_sb = [pc.tile([128, E], f16, tag=f"wq{e}", name=f"wq{e}") for e in range(EC)]
            xq_sb = [pc.tile([128, NL], f16, tag=f"xq{e}", name=f"xq{e}") for e in range(EC)]
            for e in range(EC):
                nc.sync.dma_start(xk_sb[e][:], xkT[e * 128:(e + 1) * 128, :])
            for e in range(EC):
                nc.sync.dma_start(wq_sb[e][:], WqT[e * 128:(e + 1) * 128, :])
            for e in range(EC):
                nc.sync.dma_start(xq_sb[e][:], xqT[e * 128:(e + 1) * 128, :])
            for e in range(EC):
                nc.sync.dma_start(xv_sb[e][:], xvT[e * 128:(e + 1) * 128, :])
            # ACT queue: k/v weights, small consts, masks, p4 weights
            wk_all = pc.tile([128, EC * KVE], f16, tag="wk_all")
            wv_all = pc.tile([128, EC * KVE], f16, tag="wv_all")
            wk_sb = [wk_all[:, e * KVE:(e + 1) * KVE] for e in range(EC)]
            wv_sb = [wv_all[:, e * KVE:(e + 1) * KVE] for e in range(EC)]
            nc.scalar.dma_start(
                wk_all[:].rearrange("p (e c) -> p e c", e=EC),
                WkT.rearrange("(e p) c -> p e c", e=EC))
            nc.scalar.dma_start(
                wv_all[:].rearrange("p (e c) -> p e c", e=EC),
                WvT.rearrange("(e p) c -> p e c", e=EC))

            mask_sb = [pp.tile([128, TILE_W[t]], f16, tag=f"mk{t}", name=f"mk{t}")
                       for t in range(6)]
            for t in range(6):
                nc.sync.dma_start(mask_sb[t][:], masks_in[t])
            bq_sb = [pc.tile([128, 1], f32, tag=f"bq{m}", name=f"bq{m}") for m in range(EC)]
            for e in range(EC):
                nc.scalar.dma_start(bq_sb[e][:], bq2d[e:e + 1, :])
            bk_sb = [pc.tile([128, 1], f32, tag=f"bk{m}", name=f"bk{m}") for m in range(2)]
            bv_sb = [pc.tile([128, 1], f32, tag=f"bv{m}", name=f"bv{m}") for m in range(2)]
            for m in range(2):
                nc.scalar.dma_start(bk_sb[m][:], bk2d[m:m + 1, :])
                nc.scalar.dma_start(bv_sb[m][:], bv2d[m:m + 1, :])
            ones_k1 = pc.tile([1, 128], f32r, tag="ones_k1")
            nc.scalar.dma_start(ones_k1[:], ones1)
            ones_m1 = pc.tile([128, 1], f16, tag="ones_m1")
            nc.scalar.dma_start(ones_m1[:], ones1h)
            ones_k1h = pc.tile([1, 128], f16, tag="ones_k1h")
            nc.scalar.dma_start(ones_k1h[:], ones1h)
            lng_sb = [pp.tile([128, 1], f32, tag=f"lng{e}", name=f"lng{e}") for e in range(EC)]
            lnb_sb = [pp.tile([128, 1], f32, tag=f"lnb{e}", name=f"lnb{e}") for e in range(EC)]
            for e in range(EC):
                nc.scalar.dma_start(lng_sb[e][:], lng[e:e + 1, :])
                nc.scalar.dma_start(lnb_sb[e][:], lnb[e:e + 1, :])
            bo_sb = pp.tile([1, E], f16, tag="bo", name="bo")
            nc.sync.dma_start(bo_sb[:], bo1)
            wo_all = pp.tile([128, EC * E], f16, tag="wo_all", name="wo_all")
            wo_sb = [wo_all[:, e * E:(e + 1) * E] for e in range(EC)]
            for e in range(EC):
                nc.sync.dma_start(wo_sb[e], WoT[e * 128:(e + 1) * 128, :])

            ident = pc.tile([128, 128], f16, tag="ident")
            from concourse.masks import make_identity
            make_identity(nc, ident[:])
            eps_c = pc.tile([1, 1], f32, tag="eps_c")
            nc.gpsimd.memset(eps_c[:], LN_EPS)
            ones64r = pc.tile([65, 64], f32r, tag="ones64r")
            nc.scalar.dma_start(ones64r[64:65, :], ones1[:, 0:64])

            # persistent activation tiles
            kT_sb = [pp.tile([128, N], f16, tag=f"kt{m}", name=f"kt{m}") for m in range(2)]
            v_ext = [pp.tile([128, 4 * 128], f16, tag=f"vx{k}", name=f"vx{k}") for k in range(SB)]
            qp_sb = [pp.tile([128, NL], f16, tag=f"qp{m}", name=f"qp{m}") for m in range(EC)]
            attT = [pp.tile([128, NL], f16, tag=f"at{e}", name=f"at{e}") for e in range(EC)]
            _EVEN = [0, 1, 2, 3, 8, 9, 10, 11]    # heads whose kv head is even
            _ODD = [4, 5, 6, 7, 12, 13, 14, 15]

            def _qslot(g):
                if (g // G) % 2 == 0:
                    return _EVEN.index(g), 0
                return _ODD.index(g), 1

            # ---------------- phase 0: k projection (xk+wk arrive first) ---
            with tc.tile_pool(name="psk2", bufs=2, space="PSUM") as psk2:
                for st in range(4):
                    sl = slice(st * 512, (st + 1) * 512)
                    for mt in range(2):
                        psk = psk2.tile([128, 512], f32, tag="psk")
                        for e in range(EC):
                            nc.tensor.matmul(
                                psk[:], wk_sb[e][:, mt * 128:(mt + 1) * 128],
                                xk_sb[e][:, sl], start=(e == 0), stop=(e == EC - 1))
                        nc.scalar.activation(kT_sb[mt][:, sl], psk[:], AF.Identity,
                                             bias=bk_sb[mt][:], scale=1.0)

            # ---------------- phase 1: q projection ----------------
            with tc.tile_pool(name="psq", bufs=2, space="PSUM") as psq:
                for mt2 in range(EC // 2):
                    psA = psq.tile([128, NL], f32, tag="psqA", name="psqA")
                    psB = psq.tile([128, NL], f32, tag="psqB", name="psqB")
                    for e in range(EC):
                        for mt, ps in ((2 * mt2, psA), (2 * mt2 + 1, psB)):
                            nc.tensor.matmul(
                                ps[:], wq_sb[e][:, mt * 128:(mt + 1) * 128],
                                xq_sb[e][:], start=(e == 0), stop=(e == EC - 1))
                    for mt, ps in ((2 * mt2, psA), (2 * mt2 + 1, psB)):
                        for t in range(2):
                            g = 2 * mt + t
                            ti, slot = _qslot(g)
                            nc.scalar.activation(
                                qp_sb[ti][slot * 64:(slot + 1) * 64, :],
                                ps[t * 64:(t + 1) * 64, :], AF.Identity,
                                bias=bq_sb[mt][t * 64:(t + 1) * 64, :], scale=1.0)

            # ---------------- phase 2: k/v projections ----------------
            with (
                tc.tile_pool(name="vt", bufs=2) as pvt,
                tc.tile_pool(name="pskv", bufs=2, space="PSUM") as pskv,
                tc.tile_pool(name="pst", bufs=2, space="PSUM") as pst,
            ):
                for st in range(4):  # s-tiles of 512
                    sl = slice(st * 512, (st + 1) * 512)
                    for mt in range(2):
                        psv = pskv.tile([128, 512], f32, tag="psv")
                        for e in range(EC):
                            nc.tensor.matmul(
                                psv[:], wv_sb[e][:, mt * 128:(mt + 1) * 128],
                                xv_sb[e][:, sl], start=(e == 0), stop=(e == EC - 1))
                        vt = pvt.tile([128, 512], f16, tag="vt")
                        nc.scalar.activation(vt[:], psv[:], AF.Identity,
                                             bias=bv_sb[mt][:], scale=1.0)
                        for ss in range(4):
                            k = st * 4 + ss
                            pt = pst.tile([128, 128], f16, tag="pt")
                            nc.tensor.transpose(pt[:], vt[:, ss * 128:(ss + 1) * 128],
                                                ident[:])
                            src = pt[:].rearrange("p (h x) -> p h x", h=2)
                            dst = v_ext[k][:].rearrange("p (h x) -> p h x", h=4)
                            nc.vector.tensor_copy(dst[:, 2 * mt:2 * mt + 2, 0:64], src)
                for k in range(SB):
                    pad = v_ext[k][:].rearrange("p (h x) -> p h x", h=4)[:, :, 64:128]
                    nc.gpsimd.memset(pad, 0.0)
                    one_col = v_ext[k][:].rearrange("p (h x) -> p h x", h=4)[:, :, 64:65]
                    nc.gpsimd.memset(one_col, 1.0)

            # ---------------- phase 3: attention ----------------
            with (
                tc.tile_pool(name="exs", bufs=5) as pex,
                tc.tile_pool(name="recs", bufs=2) as prec,
                tc.tile_pool(name="psim", bufs=3, space="PSUM") as psim,
                tc.tile_pool(name="pspv", bufs=1, space="PSUM") as pspv,
                tc.tile_pool(name="psbc", bufs=1, space="PSUM") as psbc,
            ):
                def head_work(g):
                    h = g // G
                    ti, slot = _qslot(g)
                    qg = qp_sb[ti][slot * 64:(slot + 1) * 64, :]
                    kh = kT_sb[h // 2][(h % 2) * 64:(h % 2) * 64 + 64, :]
                    pv = pspv.tile([128, 512], f32, tag="pv", name="pv")
                    exs = [None] * 6

                    def do_sim(t):
                        st_ = psim.tile([128, TILE_W[t]], f32, tag="sim", name="sim")
                        for k in TILE_BLOCKS[t]:
                            _, off, n0, w = BLK[k]
                            nc.tensor.matmul(
                                st_[:, off:off + w],
                                kh[:, k * 128:(k + 1) * 128], qg[:, n0:512],
                                start=True, stop=True)
                        ex = pex.tile([128, TILE_W[t]], f16, tag="ex", name="ex")
                        exs[t] = ex
                        for lo, hi in SPANS[t]:
                            nc.scalar.activation(ex[:, lo:hi], st_[:, lo:hi], AF.Exp)
                            nc.vector.tensor_tensor(
                                ex[:, lo:hi], ex[:, lo:hi], mask_sb[t][:, lo:hi],
                                op=OP.mult)

                    def do_pv(t):
                        ex = exs[t]
                        for k in TILE_BLOCKS[t]:
                            _, off, n0, w = BLK[k]
                            nc.tensor.matmul(
                                pv[:, n0:512], v_ext[k][:, 128 * h:128 * h + 128],
                                ex[:, off:off + w],
                                start=(k == 0), stop=(k == SB - 1),
                                skip_group_check=True)

                    def den_copy():
                        # den row (partition 64) to SBUF right after PV ends
                        den = prec.tile([65, 512], f32r, tag="den", name="den")
                        nc.vector.tensor_copy(den[64:65, :], pv[64:65, :])
                        return den

                    def tail(den):
                        # issued one head later so the PE's bc matmul never waits
                        bc = psbc.tile([64, 512], f32, tag="bc", name="bc")
                        nc.tensor.matmul(bc[:], ones64r[64:65, :], den[64:65, :],
                                         start=True, stop=True)
                        rec = prec.tile([64, 512], f32, tag="rec", name="rec")
                        nc.vector.reciprocal_approx_fast(rec[:], bc[:])
                        p0 = (g % 2) * 64
                        att = attT[g // 2][p0:p0 + 64, :]
                        nc.vector.tensor_tensor(att, pv[0:64, :], rec[:], op=OP.mult)

                    return do_sim, do_pv, den_copy, tail

                prev_tail = None
                for g in range(HQ):
                    do_sim, do_pv, den_copy, tail = head_work(g)
                    # software pipeline: sim runs 2-3 tiles ahead of pv;
                    # previous head's tail issues after this head's first sims.
                    do_sim(0)
                    do_sim(1)
                    if prev_tail is not None:
                        prev_tail()
                    do_sim(2)
                    do_sim(3)
                    do_pv(0)
                    do_pv(1)
                    do_sim(4)
                    do_pv(2)
                    do_pv(3)
                    do_sim(5)
                    do_pv(4)
                    do_pv(5)
                    den = den_copy()
                    prev_tail = (lambda t=tail, d=den: t(d))
                prev_tail()

            # ---------------- phase 4: layernorm + out projection ----------------
            with (
                tc.tile_pool(name="lnt", bufs=2) as plnt,
                tc.tile_pool(name="ysb", bufs=2) as pysb,
                tc.tile_pool(name="psst", bufs=1, space="PSUM") as psst,
                tc.tile_pool(name="pslb", bufs=1, space="PSUM") as pslb,
                tc.tile_pool(name="psy", bufs=2, space="PSUM") as psy,
            ):
                st_sum = psst.tile([1, NL], f32, tag="ssum")
                st_sq = psst.tile([1, NL], f32, tag="ssq")
                for e in range(EC):
                    nc.tensor.matmul(st_sum[:], ones_m1[:], attT[e][:],
                                     start=(e == 0), stop=(e == EC - 1))
                    sq = plnt.tile([128, NL], f16, tag="sq")
                    nc.scalar.activation(sq[:], attT[e][:], AF.Square)
                    nc.tensor.matmul(st_sq[:], ones_m1[:], sq[:],
                                     start=(e == 0), stop=(e == EC - 1))
                mu = plnt.tile([1, NL], f32r, tag="mu")
                nc.vector.tensor_scalar_mul(mu[:], st_sum[:], 1.0 / E)
                var = plnt.tile([1, NL], f32, tag="var")
                nc.vector.tensor_scalar_mul(var[:], st_sq[:], 1.0 / E)
                mu2 = plnt.tile([1, NL], f32, tag="mu2")
                nc.vector.tensor_tensor(mu2[:], mu[:], mu[:], op=OP.mult)
                nc.vector.tensor_tensor(var[:], var[:], mu2[:], op=OP.subtract)
                sd = plnt.tile([1, NL], f32, tag="sd")
                nc.scalar.activation(sd[:], var[:], AF.Sqrt, bias=eps_c[:])
                nc.vector.reciprocal_approx_fast(sd[:], sd[:])
                sdr = plnt.tile([1, NL], f32r, tag="sdr")
                nc.vector.tensor_copy(sdr[:], sd[:])
                mb = pslb.tile([128, NL], f32, tag="mb")
                nc.tensor.matmul(mb[:], ones_k1[:], mu[:], start=True, stop=True)
                ib = pslb.tile([128, NL], f32, tag="ib")
                nc.tensor.matmul(ib[:], ones_k1[:], sdr[:], start=True, stop=True)
                for e in range(EC):
                    tmp = plnt.tile([128, NL], f32, tag="xn")
                    nc.vector.tensor_tensor(tmp[:], attT[e][:], mb[:], op=OP.subtract)
                    nc.vector.tensor_tensor(tmp[:], tmp[:], ib[:], op=OP.mult)
                    nc.vector.tensor_scalar(attT[e][:], tmp[:], lng_sb[e][:],
                                            lnb_sb[e][:], op0=OP.mult, op1=OP.add)
                for nt in range(4):
                    pyA = psy.tile([128, 512], f32, tag="pyA", name="pyA")
                    pyB = psy.tile([128, 512], f32, tag="pyB", name="pyB")
                    pys = (pyA, pyB)
                    for e in range(EC):
                        for oc in range(2):
                            nc.tensor.matmul(
                                pys[oc][:], attT[e][:, nt * 128:(nt + 1) * 128],
                                wo_sb[e][:, oc * 512:(oc + 1) * 512],
                                start=(e == 0), stop=False)
                    for oc in range(2):
                        nc.tensor.matmul(pys[oc][:], ones_k1h[:],
                                         bo_sb[0:1, oc * 512:(oc + 1) * 512],
                                         start=False, stop=True)
                        ys = pysb.tile([128, 512], f32, tag="ys")
                        nc.vector.tensor_copy(ys[:], pys[oc][:])
                        nc.sync.dma_start(
                            y[nt * 128:(nt + 1) * 128, oc * 512:(oc + 1) * 512],
                            ys[:])
    nc.finalize()
    return nc


def _build_masks(adjc):
    """adjc: [NL, N] int (causal&adj premasked). Returns 6 f16 tiles in the
    block layout (tile cols <-> (block, n-range))."""
    at = adjc.T.astype(np.float16)  # [N, NL] = [s, n]
    tiles = [np.zeros((128, TILE_W[t]), np.float16) for t in range(6)]
    for k in range(SB):
        t, off, n0, w = BLK[k]
        tiles[t][:, off:off + w] = at[k * 128:(k + 1) * 128, n0:512]
    return tiles


def shard_inputs(inputs):
    q = np.asarray(inputs["query"], np.float32)
    k = np.asarray(inputs["key"], np.float32)
    v = np.asarray(inputs["value"], np.float32)
    adj = np.asarray(inputs["adj"], np.int32)
    WqT8 = (np.ascontiguousarray(np.asarray(inputs["Wq"], np.float32).T)
            / np.float32(8.0)).astype(np.float16)
    WkT = np.ascontiguousarray(np.asarray(inputs["Wk"], np.float32).T).astype(np.float16)
    WvT = np.ascontiguousarray(np.asarray(inputs["Wv"], np.float32).T).astype(np.float16)
    WoT = np.ascontiguousarray(np.asarray(inputs["Wo"], np.float32).T).astype(np.float16)
    bq8 = (np.asarray(inputs["bq"], np.float32) / np.float32(8.0)).reshape(EC, 128)
    bk2 = np.asarray(inputs["bk"], np.float32).reshape(2, 128)
    bv2 = np.asarray(inputs["bv"], np.float32).reshape(2, 128)
    bo1 = np.asarray(inputs["bo"], np.float32).reshape(1, E).astype(np.float16)
    lng = np.asarray(inputs["ln_g"], np.float32).reshape(EC, 128)
    lnb = np.asarray(inputs["ln_b"], np.float32).reshape(EC, 128)

    shared = dict(WqT=WqT8, WkT=WkT, WvT=WvT, WoT=WoT, bq2d=bq8, bk2d=bk2,
                  bv2d=bv2, bo1=bo1, lng=lng, lnb=lnb,
                  ones1=np.ones((1, 128), np.float32),
                  ones1h=np.ones((1, 128), np.float16))
    per_b = []
    s_idx = np.arange(N)
    for b in range(B):
        per_b.append((np.ascontiguousarray(k[b].T).astype(np.float16),
                      np.ascontiguousarray(v[b].T).astype(np.float16)))
    in_maps = []
    for c in range(8):
        b, j = divmod(c, 4)
        rows = np.arange(j, N, 4)
        causal = s_idx[None, :] <= rows[:, None]          # [NL, N]
        adjc = np.where(causal, adj[b][rows], 0)
        m = dict(shared)
        m["xqT"] = np.ascontiguousarray(q[b][rows].T).astype(np.float16)
        m["xkT"], m["xvT"] = per_b[b]
        for t, mk in enumerate(_build_masks(adjc)):
            m[f"mask{t}"] = mk
        in_maps.append(m)
    return in_maps


def _host_fixup(out, inputs):
    """Rows with no unmasked causal position get the reference's uniform-
    softmax-over-everything fallback, computed exactly on host."""
    adj = np.asarray(inputs["adj"])
    s_idx = np.arange(N)
    causal = s_idx[None, :] <= s_idx[:, None]
    for b in range(B):
        amr = np.where(((adj[b] != 0) & causal).sum(1) == 0)[0]
        if len(amr) == 0:
            continue
        v = np.asarray(inputs["value"][b], np.float64)
        Wv = np.asarray(inputs["Wv"], np.float64)
        bv = np.asarray(inputs["bv"], np.float64)
        vp = v @ Wv.T + bv                       # [N, KVE]
        mv = vp.mean(0)                          # [KVE]
        row = np.concatenate([mv[(k // G) * D:(k // G) * D + D] for k in range(HQ)])
        mu = row.mean()
        var = ((row - mu) ** 2).mean()
        rown = (row - mu) / np.sqrt(var + LN_EPS)
        rown = rown * np.asarray(inputs["ln_g"], np.float64) + np.asarray(
            inputs["ln_b"], np.float64)
        yrow = rown @ np.asarray(inputs["Wo"], np.float64).T + np.asarray(
            inputs["bo"], np.float64)
        out[b, amr, :] = yrow.astype(np.float32)
    return out


def unshard_outputs(results, inputs):
    out = np.empty((B, N, E), np.float32)
    for c in range(8):
        b, j = divmod(c, 4)
        out[b, j::4, :] = results[c]["y"]
    return _host_fixup(out, inputs)


def kernel(**inputs):
    from concourse.bass_utils import run_bass_kernel_spmd

    if "nc" not in _PROG_CACHE:
        _PROG_CACHE["nc"] = build_program()
    nc = _PROG_CACHE["nc"]
    in_maps = shard_inputs(inputs)
    res = run_bass_kernel_spmd(nc, in_maps, core_ids=list(range(8)))
    return unshard_outputs(res.results, inputs)
